# revision 56
# baseline (speedup 1.0000x reference)
"""Trainium2 Bass kernel for the DKT (graph-based knowledge tracing) model.

Sharding across the 8 NeuronCores:
  - GCN phase: row-shard of the three [5000,5000] adjacency matmuls (625 rows
    per core), with AllGathers of the small [5000,EMB] intermediates.
  - x@ques / GRU / logit heads: data-parallel over batch (8 sequences/core).

All layouts are chosen so every matmul contracts over the partition dim with
contiguous DMA: the host passes A.T column-shards, x.T (time-major columns)
shards, host-computed layer-1 GCN activations, and transposed weights.
"""

import numpy as np

Q = 2500
NQ = 5000
EMB = 128
H = 128
B = 64
L = 200
NCORES = 8
SHARD = NQ // NCORES          # 625 adjacency rows per core
KC = 125                      # contraction chunk (partition dim)
NK = NQ // KC                 # 40 chunks
BLOC = B // NCORES            # 8 sequences per core
BLC = L * BLOC                # 1600 (bl index = t*8 + b, t-major)
SHARD_P = 640                 # shard padded to even halves (fp32r ISA rule)
NH = [(0, 320), (320, 320)]   # padded-shard column halves (>=256, even)
XNT = [(0, 512), (512, 512), (1024, 512), (1536, 64)]  # x-stage N tiles
HNT = [(0, 512), (512, 512), (1024, 512), (1536, 512), (2048, 452)]

_BUILT = None
LAST = None


def _build(debug=False):
    import concourse.bass as bass  # noqa: F401
    import concourse.tile as tile
    from concourse import bacc, mybir
    from concourse.masks import make_identity
    from contextlib import ExitStack

    f32 = mybir.dt.float32
    f32r = mybir.dt.float32r
    bf16 = mybir.dt.bfloat16
    AFT = mybir.ActivationFunctionType
    ALU = mybir.AluOpType

    nc = bacc.Bacc("TRN2", target_bir_lowering=False, debug=False,
                   num_devices=NCORES)

    def din(name, shape, dt=f32r):
        return nc.dram_tensor(name, shape, dt, kind="ExternalInput").ap()

    def dout(name, shape, dt=f32):
        return nc.dram_tensor(name, shape, dt, kind="ExternalOutput").ap()

    # --- inputs (per-core unless noted) ---
    # at/xt/z1 come host-permuted to k-chunk-contiguous per partition:
    # arr2[p, k*W + j] = arr[k*KC + p, j] so every DMA is a plain
    # contiguous [125, N] slice (SWDGE descriptor-gen cost was a
    # bottleneck for the strided "(k p) -> p k" views).
    at = {g: din(f"at_{g}", [KC, NK * SHARD_P], bf16)
          for g in ("hg", "g1", "g2")}
    xt = din("xt", [KC, NK * BLC], bf16)
    z1 = {g: din(f"z1_{g}", [KC, NK * EMB], bf16) for g in ("hg", "g1", "g2")}
    e2s = {"hg": EMB, "g1": EMB // 2, "g2": EMB // 2}
    w2 = {g: din(f"w2_{g}", [EMB, e2s[g]], bf16) for g in ("hg", "g1", "g2")}
    b2 = {g: din(f"b2_{g}", [1, e2s[g]], bf16) for g in ("hg", "g1", "g2")}
    wihT = [din("wihT1", [EMB, 3 * H], bf16), din("wihT2", [EMB, 3 * H], bf16)]
    whhT = [din("whhT1", [EMB, 3 * H], bf16), din("whhT2", [EMB, 3 * H], bf16)]
    projb = [din("projb1", [EMB, 3], f32), din("projb2", [EMB, 3], f32)]
    bhhn = [din("bhhn1", [1, EMB], bf16), din("bhhn2", [1, EMB], bf16)]
    w1wT = din("w1wT", [EMB, EMB], bf16)
    w2wT = din("w2wT", [EMB, EMB], bf16)
    wb = din("wb", [EMB, 1], f32)
    fccwT = din("fccwT", [EMB, Q], bf16)
    fctwT = din("fctwT", [EMB, Q], bf16)
    fcewT = din("fcewT", [2 * EMB, Q], bf16)

    out_c = dout("out_c", [L, BLOC, Q], bf16)
    out_t = dout("out_t", [L, BLOC, Q], bf16)
    out_e = dout("out_e", [L, BLOC, Q], bf16)
    dbg = {}
    if debug:
        dbg["qh"] = dout("dbg_qh", [NQ, EMB], bf16)
        dbg["qd"] = dout("dbg_qd", [NQ, EMB], bf16)
        dbg["xh"] = dout("dbg_xh", [EMB, BLC], bf16)
        dbg["xd"] = dout("dbg_xd", [EMB, BLC], bf16)
        dbg["xp"] = dout("dbg_xp", [EMB, L * 48], bf16)
        dbg["outT"] = dout("dbg_outT", [EMB, L * 16], bf16)

    with tile.TileContext(nc) as tc, ExitStack() as ctx:
        const = ctx.enter_context(tc.tile_pool(name="const", bufs=1))
        dram = ctx.enter_context(tc.tile_pool(name="dram", bufs=1, space="DRAM"))

        ident = const.tile([128, 128], f32, name="ident")
        make_identity(nc, ident[:])
        ones_f = const.tile([1, 128], f32, name="ones_f")
        nc.gpsimd.memset(ones_f[:], 1.0)
        ones = const.tile([1, 128], bf16, name="ones")
        nc.vector.tensor_copy(ones[:], ones_f[:])
        ident_r = const.tile([128, 128], f32r, name="ident_r")
        nc.vector.tensor_copy(ident_r[:], ident[:])

        # DRAM bounce buffers for the AllGathers
        zb = {"hg": dram.tile([SHARD, EMB], bf16, name="zb_hg"),
              "pr": dram.tile([SHARD, EMB], bf16, name="zb_pr")}
        zf = {"hg": dram.tile([NQ, EMB], bf16, name="zf_hg", addr_space="Shared"),
              "pr": dram.tile([NQ, EMB], bf16, name="zf_pr", addr_space="Shared")}
        qb = {"hg": dram.tile([SHARD, EMB], bf16, name="qb_hg"),
              "pr": dram.tile([SHARD, EMB], bf16, name="qb_pr")}
        qf = {"hg": dram.tile([NQ, EMB], bf16, name="qf_hg", addr_space="Shared"),
              "pr": dram.tile([NQ, EMB], bf16, name="qf_pr", addr_space="Shared")}
        RG = [list(range(NCORES))]

        def allgather(inb, outb):
            nc.gpsimd.collective_compute(
                "AllGather", ALU.bypass, replica_groups=RG,
                ins=[inb.opt()], outs=[outb.opt()])

        def rearr_kpe(ap, e):
            return ap.rearrange("(k p) e -> p k e", p=KC)

        AHALF = NK // 2 * SHARD_P          # columns per half-pass A tile

        # SWDGE (gpsimd) sprays one dma_start across all 16 SDMA engines;
        # a sync-queue (HWDGE) dma runs on a single engine (~36 GB/s) and
        # serializes. All bulk streams go through gpsimd.
        def big_dma(out, in_):
            nc.gpsimd.dma_start(out, in_)

        # ================= GCN phase =================
        # x-phase pools are allocated UP FRONT (disjoint SBUF/PSUM from the
        # GCN pools) so the x@qh pass can overlap the GCN tail instead of
        # inheriting write-after-read deps from address reuse.
        sbQ = ctx.enter_context(tc.tile_pool(name="sbQ", bufs=1))
        qh_sb = sbQ.tile([KC, NK * EMB], bf16, name="qh_sb")
        qd_sb = sbQ.tile([KC, NK * EMB], bf16, name="qd_sb")
        sbP = ctx.enter_context(tc.tile_pool(name="sbP", bufs=1))
        xp = sbP.tile([EMB, L * 48], bf16, name="xp")
        xp_v = xp.rearrange("p (t u g b) -> p t u g b", u=2, g=3, b=BLOC)
        sbX = ctx.enter_context(tc.tile_pool(name="sbX", bufs=1))
        xhT = sbX.tile([EMB, BLC], bf16, name="xhT")
        xdT = sbX.tile([EMB, BLC], bf16, name="xdT")
        xstream = ctx.enter_context(tc.tile_pool(name="xstream", bufs=2))
        psX_cm = tc.tile_pool(name="psX", bufs=1, space="PSUM")
        psX = psX_cm.__enter__()
        with tc.tile_pool(name="sbG", bufs=1) as sbG, \
             tc.tile_pool(name="astream", bufs=2) as astream, \
             tc.tile_pool(name="psA", bufs=2, space="PSUM") as psA, \
             tc.tile_pool(name="psW", bufs=1, space="PSUM") as psW, \
             tc.tile_pool(name="psT", bufs=1, space="PSUM") as psT:

            z1sb, hT, w2sb, b2sb = {}, {}, {}, {}
            zstag, qstag, z2f = {}, {}, {}

            def gcn_stage1(g):
                e2 = e2s[g]
                z1sb[g] = sbG.tile([KC, NK * EMB], bf16, name=f"z1sb_{g}", tag="z1sb", bufs=2)
                big_dma(z1sb[g][:], z1[g][:])
                w2sb[g] = sbG.tile([EMB, e2], bf16, name=f"w2sb_{g}")
                nc.sync.dma_start(w2sb[g][:], w2[g][:])
                b2sb[g] = sbG.tile([1, e2], bf16, name=f"b2sb_{g}")
                nc.sync.dma_start(b2sb[g][:], b2[g][:])
                hT[g] = sbG.tile([EMB, SHARD_P], bf16, name=f"hT_{g}", tag="hT", bufs=2)

                ps = [psA.tile([EMB, 512], f32, name=f"ps1_{g}{i}", tag="psA")
                      for i in range(2)]
                for h in range(2):
                    a_t = astream.tile([KC, AHALF], bf16, name="a_t", tag="a")
                    big_dma(a_t[:], at[g][:, h * AHALF:(h + 1) * AHALF])
                    for kk in range(NK // 2):
                        k = h * (NK // 2) + kk
                        for i, (off, nh) in enumerate(NH):
                            nc.tensor.matmul(
                                ps[i][:, :nh],
                                z1sb[g][:, k * EMB:(k + 1) * EMB],
                                a_t[:, kk * SHARD_P + off:kk * SHARD_P + off + nh],
                                start=(k == 0), stop=(k == NK - 1))
                for i, (off, nh) in enumerate(NH):
                    nc.scalar.activation(hT[g][:, off:off + nh],
                                         ps[i][:EMB, :nh], AFT.Relu)

            def gcn_stage2w(g, grp, coloff):
                # Z2 = h @ W2 + b2 (natural layout, into the AG input staging)
                e2 = e2s[g]
                if grp not in zstag:
                    zstag[grp] = sbG.tile([KC, 5 * EMB], bf16,
                                          name=f"zstag_{grp}")
                for c in range(5):
                    ps = psW.tile([KC, EMB], f32, name="psW", tag="psW")
                    nc.tensor.matmul(ps[:, :e2], hT[g][:, c * KC:(c + 1) * KC],
                                     w2sb[g][:], start=True, stop=False)
                    nc.tensor.matmul(ps[:, :e2], ones[:, :KC], b2sb[g][:],
                                     start=False, stop=True)
                    nc.vector.tensor_copy(
                        zstag[grp][:, c * EMB + coloff: c * EMB + coloff + e2],
                        ps[:, :e2])

            def ag_z(grp):
                nc.sync.dma_start(
                    zb[grp].rearrange("(c p) e -> p c e", p=KC),
                    zstag[grp].rearrange("p (c e) -> p c e", c=5))
                allgather(zb[grp], zf[grp])
                z2f[grp] = sbG.tile([KC, NK * EMB], bf16, name=f"z2f_{grp}", tag="z2f", bufs=2)
                nc.gpsimd.dma_start(
                    z2f[grp].rearrange("p (k e) -> p k e", k=NK),
                    rearr_kpe(zf[grp], EMB))

            def gcn_stage2a(g, grp, coloff):
                e2 = e2s[g]
                o2T = sbG.tile([e2, SHARD_P], f32, name=f"o2T_{g}", tag="o2T", bufs=2)
                ps = [psA.tile([EMB, 512], f32, name=f"ps2_{g}{i}", tag="psA")
                      for i in range(2)]
                for h in range(2):
                    a_t = astream.tile([KC, AHALF], bf16, name="a_t2", tag="a")
                    big_dma(a_t[:], at[g][:, h * AHALF:(h + 1) * AHALF])
                    for kk in range(NK // 2):
                        k = h * (NK // 2) + kk
                        for i, (off, nh) in enumerate(NH):
                            nc.tensor.matmul(
                                ps[i][:e2, :nh],
                                z2f[grp][:, k * EMB + coloff: k * EMB + coloff + e2],
                                a_t[:, kk * SHARD_P + off:kk * SHARD_P + off + nh],
                                start=(k == 0), stop=(k == NK - 1))
                for i, (off, nh) in enumerate(NH):
                    nc.vector.tensor_copy(o2T[:, off:off + nh], ps[i][:e2, :nh])
                # transpose to natural layout; stage for the output AllGather
                if grp not in qstag:
                    qstag[grp] = sbG.tile([KC, 5 * EMB], bf16,
                                          name=f"qstag_{grp}")
                # ques_d = concat([ques_in(g2), ques_out(g1)]): g2 -> cols
                # 0:64, g1 -> cols 64:128 of each block; hg -> full block.
                qoff = {"hg": 0, "g1": 64, "g2": 0}[g]
                for c in range(5):
                    pst = psT.tile([KC, EMB], f32, name="psT", tag="psT")
                    nc.tensor.transpose(pst[:, :e2],
                                        o2T[:, c * KC:(c + 1) * KC],
                                        ident[:e2, :e2])
                    nc.vector.tensor_copy(
                        qstag[grp][:, c * EMB + qoff: c * EMB + qoff + e2],
                        pst[:, :e2])

            def ag_q(grp):
                nc.sync.dma_start(
                    qb[grp].rearrange("(c p) e -> p c e", p=KC),
                    qstag[grp].rearrange("p (c e) -> p c e", c=5))
                allgather(qb[grp], qf[grp])

            with nc.named_scope("gcn"):
                gcn_stage1("hg")
                gcn_stage2w("hg", "hg", 0)
                ag_z("hg")
                gcn_stage1("g1")
                gcn_stage2w("g1", "pr", 0)
                gcn_stage1("g2")
                gcn_stage2w("g2", "pr", 64)
                ag_z("pr")
                gcn_stage2a("hg", "hg", 0)
                ag_q("hg")
                nc.gpsimd.dma_start(qh_sb.rearrange("p (k e) -> p k e", k=NK),
                                    rearr_kpe(qf["hg"], EMB))
                gcn_stage2a("g1", "pr", 0)
                gcn_stage2a("g2", "pr", 64)
                ag_q("pr")
                nc.gpsimd.dma_start(qd_sb.rearrange("p (k e) -> p k e", k=NK),
                                    rearr_kpe(qf["pr"], EMB))

        if debug:
            nc.sync.dma_start(dbg["qh"][:], qf["hg"][:])
            nc.sync.dma_start(dbg["qd"][:], qf["pr"][:])

        # ================= x @ ques phase =================
        # two passes (x@qh then x@qd), each streaming xt afresh: pass-h only
        # needs the hg AllGather, so it overlaps the pr-group GCN tail.
        XQ = NK // 5          # 8 k-chunks per stream piece
        with nc.named_scope("xques"):
            psx = [psX.tile([EMB, 512], f32, name=f"psx{i}", tag=f"psx{i}")
                   for i in range(4)]
            for q in range(5):
                xsb = xstream.tile([KC, XQ * BLC], bf16, name="xsb",
                                   tag="xsb")
                big_dma(xsb[:], xt[:, q * XQ * BLC:(q + 1) * XQ * BLC])
                for kk in range(XQ):
                    k = q * XQ + kk
                    for i, (off, nn_) in enumerate(XNT):
                        nc.tensor.matmul(psx[i][:, :nn_],
                                         qh_sb[:, k * EMB:(k + 1) * EMB],
                                         xsb[:, kk * BLC + off:kk * BLC + off + nn_],
                                         start=(k == 0), stop=(k == NK - 1))
            for i, (off, nn_) in enumerate(XNT):
                nc.vector.tensor_copy(xhT[:, off:off + nn_], psx[i][:, :nn_])
            psx = [psX.tile([EMB, 512], f32, name=f"psx{i}b", tag=f"psx{i}")
                   for i in range(4)]
            for q in range(5):
                xsb = xstream.tile([KC, XQ * BLC], bf16, name="xsb2",
                                   tag="xsb")
                big_dma(xsb[:], xt[:, q * XQ * BLC:(q + 1) * XQ * BLC])
                for kk in range(XQ):
                    k = q * XQ + kk
                    for i, (off, nn_) in enumerate(XNT):
                        nc.tensor.matmul(psx[i][:, :nn_],
                                         qd_sb[:, k * EMB:(k + 1) * EMB],
                                         xsb[:, kk * BLC + off:kk * BLC + off + nn_],
                                         start=(k == 0), stop=(k == NK - 1))
            for i, (off, nn_) in enumerate(XNT):
                nc.vector.tensor_copy(xdT[:, off:off + nn_], psx[i][:, :nn_])

        if debug:
            nc.sync.dma_start(dbg["xh"][:], xhT[:])
            nc.sync.dma_start(dbg["xd"][:], xdT[:])
        psX_cm.__exit__(None, None, None)

        # ============ GRU input projections ============
        # xp column layout per step t: [xr1 xr2 xz1 xz2 xn1 xn2] (8 each)
        with tc.tile_pool(name="psP", bufs=3, space="PSUM") as psP, \
             tc.tile_pool(name="sbW", bufs=1) as sbW, \
             nc.named_scope("proj"):
            wih_sb, pb_sb = [], []
            for u in range(2):
                wt = sbW.tile([EMB, 3 * H], bf16, name=f"wihsb{u}")
                nc.sync.dma_start(wt[:], wihT[u][:])
                wih_sb.append(wt)
                pb = sbW.tile([EMB, 3], f32, name=f"pbsb{u}")
                nc.sync.dma_start(pb[:], projb[u][:])
                pb_sb.append(pb)
            for u in range(2):
                src = xhT if u == 0 else xdT
                for g in range(3):
                    for nt in range(4):
                        ps = psP.tile([EMB, 400], f32, name="psP",
                                      tag="psP")
                        nc.tensor.matmul(
                            ps[:], wih_sb[u][:, g * H:(g + 1) * H],
                            src[:, nt * 400:(nt + 1) * 400],
                            start=True, stop=True)
                        nc.scalar.activation(
                            xp_v[:, nt * 50:(nt + 1) * 50, u, g, :],
                            ps.rearrange("p (t b) -> p t b", b=BLOC),
                            AFT.Identity, bias=pb_sb[u][:, g:g + 1])
        if debug:
            nc.sync.dma_start(dbg["xp"][:], xp[:])

        # ================= GRU + heads phase =================
        with tc.tile_pool(name="sbR", bufs=1) as sbR, \
             tc.tile_pool(name="sbh", bufs=2) as sbh, \
             tc.tile_pool(name="sbstep", bufs=4) as sbs, \
             tc.tile_pool(name="stg", bufs=2) as stg, \
             tc.tile_pool(name="psG", bufs=5, space="PSUM") as psG, \
             tc.tile_pool(name="psTh", bufs=1, space="PSUM") as psTh, \
             tc.tile_pool(name="psH", bufs=2, space="PSUM") as psH:
            whh_sb, bhhn_sb = [], []
            for u in range(2):
                wt = sbR.tile([EMB, 3 * H], bf16, name=f"whhsb{u}")
                nc.sync.dma_start(wt[:], whhT[u][:])
                whh_sb.append(wt)
                bt = sbR.tile([1, EMB], bf16, name=f"bhhnsb{u}")
                nc.sync.dma_start(bt[:], bhhn[u][:])
                bhhn_sb.append(bt)
            w1w_sb = sbR.tile([EMB, EMB], bf16, name="w1wsb")
            nc.sync.dma_start(w1w_sb[:], w1wT[:])
            w2w_sb = sbR.tile([EMB, EMB], bf16, name="w2wsb")
            nc.sync.dma_start(w2w_sb[:], w2wT[:])
            wb_sb = sbR.tile([EMB, 1], f32, name="wbsb")
            nc.sync.dma_start(wb_sb[:], wb[:])
            hw_sb = {}
            for nm, t_ in (("fcc", fccwT), ("fct", fctwT)):
                w_ = sbR.tile([EMB, Q], bf16, name=f"{nm}wsb")
                nc.sync.dma_start(w_[:], t_[:])
                hw_sb[nm] = w_
            fce0 = sbR.tile([EMB, Q], bf16, name="fce0sb")
            nc.sync.dma_start(fce0[:], fcewT[0:EMB, :])
            fce1 = sbR.tile([EMB, Q], bf16, name="fce1sb")
            nc.sync.dma_start(fce1[:], fcewT[EMB:2 * EMB, :])

            outT = sbR.tile([EMB, L * 16], bf16, name="outT")
            outT_v = outT.rearrange("p (t u b) -> p t u b", u=2, b=BLOC)
            zero16_f = sbR.tile([EMB, 16], f32, name="zero16_f")
            nc.gpsimd.memset(zero16_f[:], 0.0)
            zero16 = sbR.tile([EMB, 16], bf16, name="zero16")
            nc.vector.tensor_copy(zero16[:], zero16_f[:])
            stag = {nm: stg.tile([128, Q], bf16, name=f"stag_{nm}")
                    for nm in ("c", "t", "e")}
            out_flat = {"c": out_c.rearrange("l b q -> (l b) q"),
                        "t": out_t.rearrange("l b q -> (l b) q"),
                        "e": out_e.rearrange("l b q -> (l b) q")}

            def head_chunk(j, nt16):
                rows = nt16 * BLOC
                lh = sbh.tile([EMB, 128], bf16, name="lh", tag="lh")
                ld = sbh.tile([EMB, 128], bf16, name="ld", tag="ld")
                nc.vector.tensor_copy(
                    lh[:, :rows].rearrange("p (t b) -> p t b", b=BLOC),
                    outT_v[:, 16 * j:16 * j + nt16, 0, :])
                nc.vector.tensor_copy(
                    ld[:, :rows].rearrange("p (t b) -> p t b", b=BLOC),
                    outT_v[:, 16 * j:16 * j + nt16, 1, :])
                pst = psTh.tile([EMB, 128], f32, name="pstheta", tag="pstheta")
                nc.tensor.matmul(pst[:, :rows], w1w_sb[:], lh[:, :rows],
                                 start=True, stop=False)
                nc.tensor.matmul(pst[:, :rows], w2w_sb[:], ld[:, :rows],
                                 start=False, stop=True)
                theta = sbh.tile([EMB, 128], bf16, name="theta", tag="theta")
                nc.scalar.activation(theta[:, :rows], pst[:, :rows],
                                     AFT.Sigmoid, bias=wb_sb[:])
                omt = sbh.tile([EMB, 128], bf16, name="omt", tag="omt")
                nc.scalar.activation(omt[:, :rows], theta[:, :rows],
                                     AFT.Identity, scale=-1.0, bias=1.0)
                od = sbh.tile([EMB, 128], bf16, name="od", tag="od")
                nc.vector.tensor_mul(od[:, :rows], theta[:, :rows],
                                     ld[:, :rows])
                oh = sbh.tile([EMB, 128], bf16, name="oh", tag="oh")
                nc.vector.tensor_mul(oh[:, :rows], omt[:, :rows],
                                     lh[:, :rows])
                for noff, nsz in HNT:
                    psc = psH.tile([128, 512], f32, name="psc", tag="psh")
                    nc.tensor.matmul(psc[:rows, :nsz], lh[:, :rows],
                                     hw_sb["fcc"][:, noff:noff + nsz],
                                     start=True, stop=True)
                    nc.scalar.activation(
                        stag["c"][:rows, noff:noff + nsz], psc[:rows, :nsz],
                        AFT.Identity)
                    psc = psH.tile([128, 512], f32, name="psc2", tag="psh")
                    nc.tensor.matmul(psc[:rows, :nsz], ld[:, :rows],
                                     hw_sb["fct"][:, noff:noff + nsz],
                                     start=True, stop=True)
                    nc.scalar.activation(
                        stag["t"][:rows, noff:noff + nsz], psc[:rows, :nsz],
                        AFT.Identity)
                    psc = psH.tile([128, 512], f32, name="psc3", tag="psh")
                    nc.tensor.matmul(psc[:rows, :nsz], od[:, :rows],
                                     fce0[:, noff:noff + nsz],
                                     start=True, stop=False)
                    nc.tensor.matmul(psc[:rows, :nsz], oh[:, :rows],
                                     fce1[:, noff:noff + nsz],
                                     start=False, stop=True)
                    nc.vector.tensor_copy(
                        stag["e"][:rows, noff:noff + nsz], psc[:rows, :nsz])
                for nm in ("c", "t", "e"):
                    nc.sync.dma_start(out_flat[nm][128 * j:128 * j + rows, :],
                                      stag[nm][:rows, :])

            gru_scope = nc.named_scope("gru")
            gru_scope.__enter__()
            for t in range(L):
                for u in range(2):
                    hp = (outT[:, 16 * (t - 1) + 8 * u:16 * (t - 1) + 8 * u + 8]
                          if t > 0 else zero16[:, 8 * u:8 * u + 8])
                    xb = 48 * t + 24 * u
                    psg = psG.tile([EMB, 24], f32, name="psg", tag="psg")
                    psrz = psg[:, 0:16]
                    psn = psg[:, 16:24]
                    nc.tensor.matmul(psrz[:, 0:8], whh_sb[u][:, 0:H], hp,
                                     start=True, stop=True)
                    nc.tensor.matmul(psrz[:, 8:16], whh_sb[u][:, H:2 * H], hp,
                                     start=True, stop=True)
                    # psn accumulates whh_n @ h + bhh_n (rank-1 ones fold)
                    nc.tensor.matmul(psn, bhhn_sb[u][:], ones[:, 0:8],
                                     start=True, stop=False)
                    nc.tensor.matmul(psn, whh_sb[u][:, 2 * H:3 * H], hp,
                                     start=False, stop=True)
                    rzp = sbs.tile([EMB, 16], f32r, name=f"rzp{u}",
                                   tag=f"rzp{u}")
                    nc.vector.tensor_add(rzp[:], psrz.bitcast(f32r),
                                         xp[:, xb:xb + 16])
                    gates = sbs.tile([EMB, 16], bf16, name=f"gates{u}",
                                     tag=f"gates{u}")
                    nc.scalar.activation(gates[:], rzp[:], AFT.Sigmoid)
                    rn = sbs.tile([EMB, 8], f32r, name=f"rn{u}", tag=f"rn{u}")
                    nc.vector.tensor_mul(rn[:], psn.bitcast(f32r),
                                         gates[:, 0:8])
                    npre = sbs.tile([EMB, 8], f32r, name=f"npre{u}",
                                    tag=f"npre{u}")
                    nc.vector.tensor_add(npre[:], rn[:],
                                         xp[:, xb + 16:xb + 24])
                    nn = sbs.tile([EMB, 8], bf16, name=f"nn{u}", tag=f"nn{u}")
                    nc.scalar.activation(nn[:], npre[:], AFT.Tanh)
                    dd = sbs.tile([EMB, 8], bf16, name=f"dd{u}", tag=f"dd{u}")
                    nc.gpsimd.tensor_sub(dd[:], hp, nn[:])
                    zd = sbs.tile([EMB, 8], bf16, name=f"zd{u}", tag=f"zd{u}")
                    nc.gpsimd.tensor_mul(zd[:], gates[:, 8:16], dd[:])
                    nc.vector.tensor_add(
                        outT[:, 16 * t + 8 * u:16 * t + 8 * u + 8],
                        nn[:], zd[:])
            gru_scope.__exit__(None, None, None)
            # heads emitted after the loop: lower scheduler priority, so the
            # recurrence chain never waits behind head matmuls
            with nc.named_scope("heads"):
                for j in range(12):
                    head_chunk(j, 16)
                head_chunk(12, 8)  # last 64 rows (t in [192,200))

            if debug:
                nc.sync.dma_start(dbg["outT"][:], outT[:])

    nc.compile()
    return nc


def _host_prep(inputs):
    """Build the 8 per-core input maps from the full problem inputs."""
    from concourse import mybir
    f = np.float32
    bf = mybir.dt.np(mybir.dt.bfloat16)
    x = inputs["x"].astype(f, copy=False)
    ques = inputs["ques"].astype(f, copy=False)

    def T(a, dt=None):
        return np.ascontiguousarray(
            np.asarray(a).T.astype(dt or f, copy=False))

    # layer-1 GCN activations, computed on host (tiny)
    z1 = {"hg": ques @ inputs["hg_W1"] + inputs["hg_b1"],
          "g1": ques @ inputs["g1_W1"] + inputs["g1_b1"],
          "g2": ques @ inputs["g2_W1"] + inputs["g2_b1"]}
    graphs = {"hg": inputs["G"], "g1": inputs["adj_out"], "g2": inputs["adj_in"]}

    def kperm(a, w):
        # [NQ, w] -> [KC, NK*w] with arr2[p, k*w+j] = arr[k*KC+p, j]
        return np.ascontiguousarray(
            np.asarray(a).reshape(NK, KC, w).transpose(1, 0, 2)
            .reshape(KC, NK * w))

    shared = {
        "z1_hg": kperm(np.asarray(z1["hg"]).astype(bf), EMB),
        "z1_g1": kperm(np.asarray(z1["g1"]).astype(bf), EMB),
        "z1_g2": kperm(np.asarray(z1["g2"]).astype(bf), EMB),
        "w2_hg": np.ascontiguousarray(np.asarray(inputs["hg_W2"]).astype(bf)),
        "w2_g1": np.ascontiguousarray(np.asarray(inputs["g1_W2"]).astype(bf)),
        "w2_g2": np.ascontiguousarray(np.asarray(inputs["g2_W2"]).astype(bf)),
        "b2_hg": np.asarray(inputs["hg_b2"]).astype(bf).reshape(1, -1),
        "b2_g1": np.asarray(inputs["g1_b2"]).astype(bf).reshape(1, -1),
        "b2_g2": np.asarray(inputs["g2_b2"]).astype(bf).reshape(1, -1),
        "wihT1": T(inputs["r1_Wih"], bf),
        "wihT2": T(inputs["r2_Wih"], bf),
        "whhT1": T(inputs["r1_Whh"], bf),
        "whhT2": T(inputs["r2_Whh"], bf),
        "w1wT": T(inputs["w1_W"], bf),
        "w2wT": T(inputs["w2_W"], bf),
        "wb": np.asarray(inputs["w1_b"] + inputs["w2_b"], f).reshape(-1, 1),
        "fccwT": T(inputs["fcc_W"], bf),
        "fctwT": T(inputs["fct_W"], bf),
        "fcewT": T(inputs["fce_W"], bf),
    }
    for u, (ih, hh) in enumerate((("r1_bih", "r1_bhh"), ("r2_bih", "r2_bhh"))):
        bih = np.asarray(inputs[ih], f)
        bhh = np.asarray(inputs[hh], f)
        pb = np.zeros((EMB, 3), f)
        for g in range(3):
            pb[:, g] = bih[g * H:(g + 1) * H]
            if g < 2:  # r, z: fold bhh into the projection bias
                pb[:, g] += bhh[g * H:(g + 1) * H]
        shared[f"projb{u + 1}"] = pb
        shared[f"bhhn{u + 1}"] = \
            bhh[2 * H:3 * H].reshape(1, -1).astype(bf).copy()

    in_maps = []
    for c in range(NCORES):
        m = dict(shared)
        for g, arr in graphs.items():
            blk = np.asarray(arr)[c * SHARD:(c + 1) * SHARD, :]
            atc = np.zeros((NQ, SHARD_P), bf)
            atc[:, :SHARD] = blk.astype(f, copy=False).T.astype(bf)
            m[f"at_{g}"] = kperm(atc, SHARD_P)
        xc = x[c * BLOC:(c + 1) * BLOC]           # [8, 200, 5000]
        m["xt"] = kperm(
            xc.transpose(2, 1, 0).reshape(NQ, BLC).astype(bf), BLC)
        in_maps.append(m)
    return in_maps


def kernel(**inputs):
    global _BUILT, LAST
    from concourse import bass_utils
    if _BUILT is None:
        _BUILT = _build(debug=False)
    nc = _BUILT
    in_maps = _host_prep(inputs)
    res = bass_utils.run_bass_kernel_spmd(nc, in_maps,
                                          core_ids=list(range(NCORES)))
    LAST = res
    f = np.float32
    logit_c = np.empty((B, L, Q), f)
    logit_t = np.empty((B, L, Q), f)
    logit_e = np.empty((B, L, Q), f)
    for c in range(NCORES):
        r = res.results[c]
        logit_c[c * BLOC:(c + 1) * BLOC] = \
            r["out_c"].astype(f).transpose(1, 0, 2)
        logit_t[c * BLOC:(c + 1) * BLOC] = \
            r["out_t"].astype(f).transpose(1, 0, 2)
        logit_e[c * BLOC:(c + 1) * BLOC] = \
            r["out_e"].astype(f).transpose(1, 0, 2)
    for arr, bname in ((logit_c, "fcc_b"), (logit_t, "fct_b"),
                       (logit_e, "fce_b")):
        bias = np.asarray(inputs[bname], f)
        if np.any(bias):
            arr += bias
    return (logit_c, logit_t, logit_e)



# revision 67
# speedup vs baseline: 1.1861x; 1.1861x over previous
"""Trainium2 Bass kernel for the DKT (graph-based knowledge tracing) model.

Sharding across the 8 NeuronCores:
  - GCN phase: row-shard of the three [5000,5000] adjacency matmuls (625 rows
    per core), with AllGathers of the small [5000,EMB] intermediates.
  - x@ques / GRU / logit heads: data-parallel over batch (8 sequences/core).

All layouts are chosen so every matmul contracts over the partition dim with
contiguous DMA: the host passes A.T column-shards, x.T (time-major columns)
shards, host-computed layer-1 GCN activations, and transposed weights.
"""

import numpy as np

Q = 2500
NQ = 5000
EMB = 128
H = 128
B = 64
L = 200
NCORES = 8
SHARD = NQ // NCORES          # 625 adjacency rows per core
KC = 125                      # contraction chunk (partition dim)
NK = NQ // KC                 # 40 chunks
BLOC = B // NCORES            # 8 sequences per core
BLC = L * BLOC                # 1600 (bl index = t*8 + b, t-major)
SHARD_P = 640                 # shard padded to even halves (fp32r ISA rule)
NH = [(0, 320), (320, 320)]   # padded-shard column halves (>=256, even)
XNT = [(0, 512), (512, 512), (1024, 512), (1536, 64)]  # x-stage N tiles
HNT = [(0, 512), (512, 512), (1024, 512), (1536, 512), (2048, 452)]

_BUILT = None
LAST = None


def _build(debug=False):
    import concourse.bass as bass  # noqa: F401
    import concourse.tile as tile
    from concourse import bacc, mybir
    from concourse.masks import make_identity
    from contextlib import ExitStack

    f32 = mybir.dt.float32
    f32r = mybir.dt.float32r
    bf16 = mybir.dt.bfloat16
    AFT = mybir.ActivationFunctionType
    ALU = mybir.AluOpType

    nc = bacc.Bacc("TRN2", target_bir_lowering=False, debug=False,
                   num_devices=NCORES)

    def din(name, shape, dt=f32r):
        return nc.dram_tensor(name, shape, dt, kind="ExternalInput").ap()

    def dout(name, shape, dt=f32):
        return nc.dram_tensor(name, shape, dt, kind="ExternalOutput").ap()

    # --- inputs (per-core unless noted) ---
    # at/xt/z1 come host-permuted to k-chunk-contiguous per partition:
    # arr2[p, k*W + j] = arr[k*KC + p, j] so every DMA is a plain
    # contiguous [125, N] slice (SWDGE descriptor-gen cost was a
    # bottleneck for the strided "(k p) -> p k" views).
    at = {g: din(f"at_{g}", [KC, NK * SHARD_P], bf16)
          for g in ("hg", "g1", "g2")}
    xt = din("xt", [KC, NK * BLC], bf16)
    z1 = {g: din(f"z1_{g}", [KC, NK * EMB], bf16) for g in ("hg", "g1", "g2")}
    e2s = {"hg": EMB, "g1": EMB // 2, "g2": EMB // 2}
    w2 = {g: din(f"w2_{g}", [EMB, e2s[g]], bf16) for g in ("hg", "g1", "g2")}
    b2 = {g: din(f"b2_{g}", [1, e2s[g]], bf16) for g in ("hg", "g1", "g2")}
    wihT = [din("wihT1", [EMB, 3 * H], bf16), din("wihT2", [EMB, 3 * H], bf16)]
    whhT = [din("whhT1", [EMB, 3 * H], bf16), din("whhT2", [EMB, 3 * H], bf16)]
    projb = [din("projb1", [EMB, 3], f32), din("projb2", [EMB, 3], f32)]
    bhhn = [din("bhhn1", [1, EMB], bf16), din("bhhn2", [1, EMB], bf16)]
    w1wT = din("w1wT", [EMB, EMB], bf16)
    w2wT = din("w2wT", [EMB, EMB], bf16)
    wb = din("wb", [EMB, 1], f32)
    fccwT = din("fccwT", [EMB, Q], bf16)
    fctwT = din("fctwT", [EMB, Q], bf16)
    fcewT = din("fcewT", [2 * EMB, Q], bf16)

    out_c = dout("out_c", [L, BLOC, Q], bf16)
    out_t = dout("out_t", [L, BLOC, Q], bf16)
    out_e = dout("out_e", [L, BLOC, Q], bf16)
    dbg = {}
    if debug:
        dbg["qh"] = dout("dbg_qh", [NQ, EMB], bf16)
        dbg["qd"] = dout("dbg_qd", [NQ, EMB], bf16)
        dbg["xh"] = dout("dbg_xh", [EMB, BLC], bf16)
        dbg["xd"] = dout("dbg_xd", [EMB, BLC], bf16)
        dbg["xp"] = dout("dbg_xp", [EMB, L * 48], bf16)
        dbg["outT"] = dout("dbg_outT", [EMB, L * 16], bf16)

    with tile.TileContext(nc) as tc, ExitStack() as ctx:
        const = ctx.enter_context(tc.tile_pool(name="const", bufs=1))
        dram = ctx.enter_context(tc.tile_pool(name="dram", bufs=1, space="DRAM"))

        ident = const.tile([128, 128], f32, name="ident")
        make_identity(nc, ident[:])
        ones_f = const.tile([1, 128], f32, name="ones_f")
        nc.gpsimd.memset(ones_f[:], 1.0)
        ones = const.tile([1, 128], bf16, name="ones")
        nc.vector.tensor_copy(ones[:], ones_f[:])
        ident_r = const.tile([128, 128], f32r, name="ident_r")
        nc.vector.tensor_copy(ident_r[:], ident[:])

        # DRAM bounce buffers for the AllGathers
        zb = {"hg": dram.tile([SHARD, EMB], bf16, name="zb_hg"),
              "pr": dram.tile([SHARD, EMB], bf16, name="zb_pr")}
        zf = {"hg": dram.tile([NQ, EMB], bf16, name="zf_hg", addr_space="Shared"),
              "pr": dram.tile([NQ, EMB], bf16, name="zf_pr", addr_space="Shared")}
        qb = {"hg": dram.tile([SHARD, EMB], bf16, name="qb_hg"),
              "pr": dram.tile([SHARD, EMB], bf16, name="qb_pr")}
        qf = {"hg": dram.tile([NQ, EMB], bf16, name="qf_hg", addr_space="Shared"),
              "pr": dram.tile([NQ, EMB], bf16, name="qf_pr", addr_space="Shared")}
        RG = [list(range(NCORES))]

        def allgather(inb, outb):
            nc.gpsimd.collective_compute(
                "AllGather", ALU.bypass, replica_groups=RG,
                ins=[inb.opt()], outs=[outb.opt()])

        def rearr_kpe(ap, e):
            return ap.rearrange("(k p) e -> p k e", p=KC)

        # SWDGE (gpsimd) sprays one dma_start across all 16 SDMA engines;
        # a sync-queue (HWDGE) dma runs on a single engine (~36 GB/s) and
        # serializes. All bulk streams go through gpsimd.
        def big_dma(out, in_):
            nc.gpsimd.dma_start(out, in_)

        # ================= GCN phase =================
        # x-phase pools are allocated UP FRONT (disjoint SBUF/PSUM from the
        # GCN pools) so the x@qh pass can overlap the GCN tail instead of
        # inheriting write-after-read deps from address reuse.
        sbQ = ctx.enter_context(tc.tile_pool(name="sbQ", bufs=1))
        qh_sb = sbQ.tile([KC, NK * EMB], bf16, name="qh_sb")
        qd_sb = sbQ.tile([KC, NK * EMB], bf16, name="qd_sb")
        sbP = ctx.enter_context(tc.tile_pool(name="sbP", bufs=1))
        xp = sbP.tile([EMB, L * 48], bf16, name="xp")
        xp_v = xp.rearrange("p (t u g b) -> p t u g b", u=2, g=3, b=BLOC)
        sbX = ctx.enter_context(tc.tile_pool(name="sbX", bufs=1))
        xhT = sbX.tile([EMB, BLC], bf16, name="xhT")
        xdT = sbX.tile([EMB, BLC], bf16, name="xdT")
        xstream = ctx.enter_context(tc.tile_pool(name="xstream", bufs=2))
        with tc.tile_pool(name="sbG", bufs=1) as sbG, \
             tc.tile_pool(name="astream", bufs=2) as astream, \
             tc.tile_pool(name="psA", bufs=2, space="PSUM") as psA, \
             tc.tile_pool(name="psW", bufs=1, space="PSUM") as psW, \
             tc.tile_pool(name="psT", bufs=1, space="PSUM") as psT:

            z1sb, hT, w2sb, b2sb = {}, {}, {}, {}
            zstag, qstag, z2f = {}, {}, {}
            ahold = {}
            APIECE = [(0, 8), (8, 16), (24, 16)]   # k-chunk DMA pieces

            def gcn_stage1(g):
                e2 = e2s[g]
                z1sb[g] = sbG.tile([KC, NK * EMB], bf16, name=f"z1sb_{g}", tag="z1sb", bufs=1)
                big_dma(z1sb[g][:], z1[g][:])
                w2sb[g] = sbG.tile([EMB, e2], bf16, name=f"w2sb_{g}")
                nc.sync.dma_start(w2sb[g][:], w2[g][:])
                b2sb[g] = sbG.tile([1, e2], bf16, name=f"b2sb_{g}")
                nc.sync.dma_start(b2sb[g][:], b2[g][:])
                hT[g] = sbG.tile([EMB, SHARD_P], bf16, name=f"hT_{g}", tag="hT", bufs=2)

                # stream at_g ONCE into a held buffer; stage2a re-reads it
                # from SBUF (at traffic halved: DMA is the phase roofline)
                a_t = astream.tile([KC, NK * SHARD_P], bf16, name=f"a_{g}",
                                   tag="a")
                ahold[g] = a_t
                for k0, nk_ in APIECE:
                    big_dma(a_t[:, k0 * SHARD_P:(k0 + nk_) * SHARD_P],
                            at[g][:, k0 * SHARD_P:(k0 + nk_) * SHARD_P])
                ps = [psA.tile([EMB, 512], f32, name=f"ps1_{g}{i}", tag="psA")
                      for i in range(2)]
                for k in range(NK):
                    for i, (off, nh) in enumerate(NH):
                        nc.tensor.matmul(
                            ps[i][:, :nh],
                            z1sb[g][:, k * EMB:(k + 1) * EMB],
                            a_t[:, k * SHARD_P + off:k * SHARD_P + off + nh],
                            start=(k == 0), stop=(k == NK - 1))
                for i, (off, nh) in enumerate(NH):
                    nc.scalar.activation(hT[g][:, off:off + nh],
                                         ps[i][:EMB, :nh], AFT.Relu)

            def gcn_stage2w(g, grp, coloff):
                # Z2 = h @ W2 + b2 (natural layout, into the AG input staging)
                e2 = e2s[g]
                if grp not in zstag:
                    zstag[grp] = sbG.tile([KC, 5 * EMB], bf16,
                                          name=f"zstag_{grp}")
                for c in range(5):
                    ps = psW.tile([KC, EMB], f32, name="psW", tag="psW")
                    nc.tensor.matmul(ps[:, :e2], hT[g][:, c * KC:(c + 1) * KC],
                                     w2sb[g][:], start=True, stop=False)
                    nc.tensor.matmul(ps[:, :e2], ones[:, :KC], b2sb[g][:],
                                     start=False, stop=True)
                    nc.vector.tensor_copy(
                        zstag[grp][:, c * EMB + coloff: c * EMB + coloff + e2],
                        ps[:, :e2])

            def ag_z(grp):
                nc.sync.dma_start(
                    zb[grp].rearrange("(c p) e -> p c e", p=KC),
                    zstag[grp].rearrange("p (c e) -> p c e", c=5))
                allgather(zb[grp], zf[grp])
                z2f[grp] = sbG.tile([KC, NK * EMB], bf16, name=f"z2f_{grp}", tag="z2f", bufs=2)
                nc.gpsimd.dma_start(
                    z2f[grp].rearrange("p (k e) -> p k e", k=NK),
                    rearr_kpe(zf[grp], EMB))

            def gcn_stage2a(g, grp, coloff):
                e2 = e2s[g]
                o2T = sbG.tile([e2, SHARD_P], f32, name=f"o2T_{g}", tag="o2T", bufs=2)
                ps = [psA.tile([EMB, 512], f32, name=f"ps2_{g}{i}", tag="psA")
                      for i in range(2)]
                a_t = ahold[g]
                for k in range(NK):
                    for i, (off, nh) in enumerate(NH):
                        nc.tensor.matmul(
                            ps[i][:e2, :nh],
                            z2f[grp][:, k * EMB + coloff: k * EMB + coloff + e2],
                            a_t[:, k * SHARD_P + off:k * SHARD_P + off + nh],
                            start=(k == 0), stop=(k == NK - 1))
                for i, (off, nh) in enumerate(NH):
                    nc.vector.tensor_copy(o2T[:, off:off + nh], ps[i][:e2, :nh])
                # transpose to natural layout; stage for the output AllGather
                if grp not in qstag:
                    qstag[grp] = sbG.tile([KC, 5 * EMB], bf16,
                                          name=f"qstag_{grp}")
                # ques_d = concat([ques_in(g2), ques_out(g1)]): g2 -> cols
                # 0:64, g1 -> cols 64:128 of each block; hg -> full block.
                qoff = {"hg": 0, "g1": 64, "g2": 0}[g]
                for c in range(5):
                    pst = psT.tile([KC, EMB], f32, name="psT", tag="psT")
                    nc.tensor.transpose(pst[:, :e2],
                                        o2T[:, c * KC:(c + 1) * KC],
                                        ident[:e2, :e2])
                    nc.vector.tensor_copy(
                        qstag[grp][:, c * EMB + qoff: c * EMB + qoff + e2],
                        pst[:, :e2])

            def ag_q(grp):
                nc.sync.dma_start(
                    qb[grp].rearrange("(c p) e -> p c e", p=KC),
                    qstag[grp].rearrange("p (c e) -> p c e", c=5))
                allgather(qb[grp], qf[grp])

            with nc.named_scope("gcn"):
                gcn_stage1("hg")
                gcn_stage2w("hg", "hg", 0)
                ag_z("hg")
                gcn_stage1("g1")
                gcn_stage2a("hg", "hg", 0)
                ag_q("hg")
                nc.gpsimd.dma_start(qh_sb.rearrange("p (k e) -> p k e", k=NK),
                                    rearr_kpe(qf["hg"], EMB))
                gcn_stage2w("g1", "pr", 0)
                gcn_stage1("g2")
                gcn_stage2w("g2", "pr", 64)
                ag_z("pr")
                gcn_stage2a("g1", "pr", 0)
                gcn_stage2a("g2", "pr", 64)
                ag_q("pr")
                nc.gpsimd.dma_start(qd_sb.rearrange("p (k e) -> p k e", k=NK),
                                    rearr_kpe(qf["pr"], EMB))

        if debug:
            nc.sync.dma_start(dbg["qh"][:], qf["hg"][:])
            nc.sync.dma_start(dbg["qd"][:], qf["pr"][:])

        # ================= x @ ques phase =================
        XQ = 2                # k-chunks per stream piece
        psX_cm = tc.tile_pool(name="psX", bufs=1, space="PSUM")
        psX = psX_cm.__enter__()
        with nc.named_scope("xques"):
            psh = [psX.tile([EMB, 512], f32, name=f"psxh{i}", tag=f"psxh{i}")
                   for i in range(4)]
            psd = [psX.tile([EMB, 512], f32, name=f"psxd{i}", tag=f"psxd{i}")
                   for i in range(4)]
            for q in range(NK // XQ):
                xsb = xstream.tile([KC, XQ * BLC], bf16, name="xsb",
                                   tag="xsb")
                big_dma(xsb[:], xt[:, q * XQ * BLC:(q + 1) * XQ * BLC])
                for kk in range(XQ):
                    k = q * XQ + kk
                    for i, (off, nn_) in enumerate(XNT):
                        nc.tensor.matmul(psh[i][:, :nn_],
                                         qh_sb[:, k * EMB:(k + 1) * EMB],
                                         xsb[:, kk * BLC + off:kk * BLC + off + nn_],
                                         start=(k == 0), stop=(k == NK - 1))
                        nc.tensor.matmul(psd[i][:, :nn_],
                                         qd_sb[:, k * EMB:(k + 1) * EMB],
                                         xsb[:, kk * BLC + off:kk * BLC + off + nn_],
                                         start=(k == 0), stop=(k == NK - 1))
            for i, (off, nn_) in enumerate(XNT):
                nc.vector.tensor_copy(xhT[:, off:off + nn_], psh[i][:, :nn_])
                nc.vector.tensor_copy(xdT[:, off:off + nn_], psd[i][:, :nn_])

        if debug:
            nc.sync.dma_start(dbg["xh"][:], xhT[:])
            nc.sync.dma_start(dbg["xd"][:], xdT[:])
        psX_cm.__exit__(None, None, None)

        # ============ GRU input projections ============
        # xp column layout per step t: [xr1 xr2 xz1 xz2 xn1 xn2] (8 each)
        with tc.tile_pool(name="psP", bufs=3, space="PSUM") as psP, \
             tc.tile_pool(name="sbW", bufs=1) as sbW, \
             nc.named_scope("proj"):
            wih_sb, pb_sb = [], []
            for u in range(2):
                wt = sbW.tile([EMB, 3 * H], bf16, name=f"wihsb{u}")
                nc.sync.dma_start(wt[:], wihT[u][:])
                wih_sb.append(wt)
                pb = sbW.tile([EMB, 3], f32, name=f"pbsb{u}")
                nc.sync.dma_start(pb[:], projb[u][:])
                pb_sb.append(pb)
            for u in range(2):
                src = xhT if u == 0 else xdT
                for g in range(3):
                    for nt in range(4):
                        ps = psP.tile([EMB, 400], f32, name="psP",
                                      tag="psP")
                        nc.tensor.matmul(
                            ps[:], wih_sb[u][:, g * H:(g + 1) * H],
                            src[:, nt * 400:(nt + 1) * 400],
                            start=True, stop=True)
                        nc.scalar.activation(
                            xp_v[:, nt * 50:(nt + 1) * 50, u, g, :],
                            ps.rearrange("p (t b) -> p t b", b=BLOC),
                            AFT.Identity, bias=pb_sb[u][:, g:g + 1])
        if debug:
            nc.sync.dma_start(dbg["xp"][:], xp[:])

        # ================= GRU + heads phase =================
        with tc.tile_pool(name="sbR", bufs=1) as sbR, \
             tc.tile_pool(name="sbh", bufs=2) as sbh, \
             tc.tile_pool(name="sbstep", bufs=4) as sbs, \
             tc.tile_pool(name="stg", bufs=2) as stg, \
             tc.tile_pool(name="psG", bufs=5, space="PSUM") as psG, \
             tc.tile_pool(name="psTh", bufs=1, space="PSUM") as psTh, \
             tc.tile_pool(name="psH", bufs=2, space="PSUM") as psH:
            whh_sb, bhhn_sb = [], []
            for u in range(2):
                wt = sbR.tile([EMB, 3 * H], bf16, name=f"whhsb{u}")
                nc.sync.dma_start(wt[:], whhT[u][:])
                whh_sb.append(wt)
                bt = sbR.tile([1, EMB], bf16, name=f"bhhnsb{u}")
                nc.sync.dma_start(bt[:], bhhn[u][:])
                bhhn_sb.append(bt)
            w1w_sb = sbR.tile([EMB, EMB], bf16, name="w1wsb")
            nc.sync.dma_start(w1w_sb[:], w1wT[:])
            w2w_sb = sbR.tile([EMB, EMB], bf16, name="w2wsb")
            nc.sync.dma_start(w2w_sb[:], w2wT[:])
            wb_sb = sbR.tile([EMB, 1], f32, name="wbsb")
            nc.sync.dma_start(wb_sb[:], wb[:])
            # head weights are not needed until ~the GRU: delay their DMA so
            # they don't steal HBM bandwidth from the GCN A-streams
            hw_sb = {}
            with tc.tile_wait_until(0.35):
                for nm, t_ in (("fcc", fccwT), ("fct", fctwT)):
                    w_ = sbR.tile([EMB, Q], bf16, name=f"{nm}wsb")
                    nc.sync.dma_start(w_[:], t_[:])
                    hw_sb[nm] = w_
                fce0 = sbR.tile([EMB, Q], bf16, name="fce0sb")
                nc.sync.dma_start(fce0[:], fcewT[0:EMB, :])
                fce1 = sbR.tile([EMB, Q], bf16, name="fce1sb")
                nc.sync.dma_start(fce1[:], fcewT[EMB:2 * EMB, :])

            outT = sbR.tile([EMB, L * 16], bf16, name="outT")
            outT_v = outT.rearrange("p (t u b) -> p t u b", u=2, b=BLOC)
            zero16_f = sbR.tile([EMB, 16], f32, name="zero16_f")
            nc.gpsimd.memset(zero16_f[:], 0.0)
            zero16 = sbR.tile([EMB, 16], bf16, name="zero16")
            nc.vector.tensor_copy(zero16[:], zero16_f[:])
            stag = {nm: stg.tile([128, Q], bf16, name=f"stag_{nm}")
                    for nm in ("c", "t", "e")}
            out_flat = {"c": out_c.rearrange("l b q -> (l b) q"),
                        "t": out_t.rearrange("l b q -> (l b) q"),
                        "e": out_e.rearrange("l b q -> (l b) q")}

            def head_chunk(j, nt16):
                rows = nt16 * BLOC
                lh = sbh.tile([EMB, 128], bf16, name="lh", tag="lh")
                ld = sbh.tile([EMB, 128], bf16, name="ld", tag="ld")
                nc.vector.tensor_copy(
                    lh[:, :rows].rearrange("p (t b) -> p t b", b=BLOC),
                    outT_v[:, 16 * j:16 * j + nt16, 0, :])
                nc.vector.tensor_copy(
                    ld[:, :rows].rearrange("p (t b) -> p t b", b=BLOC),
                    outT_v[:, 16 * j:16 * j + nt16, 1, :])
                pst = psTh.tile([EMB, 128], f32, name="pstheta", tag="pstheta")
                nc.tensor.matmul(pst[:, :rows], w1w_sb[:], lh[:, :rows],
                                 start=True, stop=False)
                nc.tensor.matmul(pst[:, :rows], w2w_sb[:], ld[:, :rows],
                                 start=False, stop=True)
                theta = sbh.tile([EMB, 128], bf16, name="theta", tag="theta")
                nc.scalar.activation(theta[:, :rows], pst[:, :rows],
                                     AFT.Sigmoid, bias=wb_sb[:])
                omt = sbh.tile([EMB, 128], bf16, name="omt", tag="omt")
                nc.scalar.activation(omt[:, :rows], theta[:, :rows],
                                     AFT.Identity, scale=-1.0, bias=1.0)
                od = sbh.tile([EMB, 128], bf16, name="od", tag="od")
                nc.vector.tensor_mul(od[:, :rows], theta[:, :rows],
                                     ld[:, :rows])
                oh = sbh.tile([EMB, 128], bf16, name="oh", tag="oh")
                nc.vector.tensor_mul(oh[:, :rows], omt[:, :rows],
                                     lh[:, :rows])
                for noff, nsz in HNT:
                    psc = psH.tile([128, 512], f32, name="psc", tag="psh")
                    nc.tensor.matmul(psc[:rows, :nsz], lh[:, :rows],
                                     hw_sb["fcc"][:, noff:noff + nsz],
                                     start=True, stop=True)
                    nc.scalar.activation(
                        stag["c"][:rows, noff:noff + nsz], psc[:rows, :nsz],
                        AFT.Identity)
                    psc = psH.tile([128, 512], f32, name="psc2", tag="psh")
                    nc.tensor.matmul(psc[:rows, :nsz], ld[:, :rows],
                                     hw_sb["fct"][:, noff:noff + nsz],
                                     start=True, stop=True)
                    nc.scalar.activation(
                        stag["t"][:rows, noff:noff + nsz], psc[:rows, :nsz],
                        AFT.Identity)
                    psc = psH.tile([128, 512], f32, name="psc3", tag="psh")
                    nc.tensor.matmul(psc[:rows, :nsz], od[:, :rows],
                                     fce0[:, noff:noff + nsz],
                                     start=True, stop=False)
                    nc.tensor.matmul(psc[:rows, :nsz], oh[:, :rows],
                                     fce1[:, noff:noff + nsz],
                                     start=False, stop=True)
                    nc.vector.tensor_copy(
                        stag["e"][:rows, noff:noff + nsz], psc[:rows, :nsz])
                for nm in ("c", "t", "e"):
                    nc.sync.dma_start(out_flat[nm][128 * j:128 * j + rows, :],
                                      stag[nm][:rows, :])

            gru_scope = nc.named_scope("gru")
            gru_scope.__enter__()
            # the two GRUs' chains are emitted op-by-op interleaved: the
            # static per-engine queue order then matches dataflow order, so
            # neither chain head-of-line-blocks the other.
            for t in range(L):
                hp, xb, psrz, psn = {}, {}, {}, {}
                psg = psG.tile([EMB, 48], f32, name="psg", tag="psg")
                for u in range(2):
                    hp[u] = (outT[:, 16 * (t - 1) + 8 * u:
                                  16 * (t - 1) + 8 * u + 8]
                             if t > 0 else zero16[:, 8 * u:8 * u + 8])
                    xb[u] = 48 * t + 24 * u
                    psrz[u] = psg[:, 24 * u:24 * u + 16]
                    psn[u] = psg[:, 24 * u + 16:24 * u + 24]
                    # bias fold is h-independent: keep PE busy at step start
                    nc.tensor.matmul(psn[u], bhhn_sb[u][:], ones[:, 0:8],
                                     start=True, stop=False)
                for u in range(2):
                    nc.tensor.matmul(psrz[u][:, 0:8], whh_sb[u][:, 0:H],
                                     hp[u], start=True, stop=True)
                    nc.tensor.matmul(psrz[u][:, 8:16], whh_sb[u][:, H:2 * H],
                                     hp[u], start=True, stop=True)
                    nc.tensor.matmul(psn[u], whh_sb[u][:, 2 * H:3 * H], hp[u],
                                     start=False, stop=True)
                rzp, gates, rn, npre, nn, dd, zd = ({} for _ in range(7))
                for u in range(2):
                    rzp[u] = sbs.tile([EMB, 16], f32r, name=f"rzp{u}",
                                      tag=f"rzp{u}")
                    nc.vector.tensor_add(rzp[u][:], psrz[u].bitcast(f32r),
                                         xp[:, xb[u]:xb[u] + 16])
                for u in range(2):
                    gates[u] = sbs.tile([EMB, 16], bf16, name=f"gates{u}",
                                        tag=f"gates{u}")
                    nc.scalar.activation(gates[u][:], rzp[u][:], AFT.Sigmoid)
                for u in range(2):
                    rn[u] = sbs.tile([EMB, 8], f32r, name=f"rn{u}",
                                     tag=f"rn{u}")
                    nc.vector.tensor_mul(rn[u][:], psn[u].bitcast(f32r),
                                         gates[u][:, 0:8])
                for u in range(2):
                    npre[u] = sbs.tile([EMB, 8], f32r, name=f"npre{u}",
                                       tag=f"npre{u}")
                    nc.vector.tensor_add(npre[u][:], rn[u][:],
                                         xp[:, xb[u] + 16:xb[u] + 24])
                for u in range(2):
                    nn[u] = sbs.tile([EMB, 8], bf16, name=f"nn{u}",
                                     tag=f"nn{u}")
                    nc.scalar.activation(nn[u][:], npre[u][:], AFT.Tanh)
                for u in range(2):
                    dd[u] = sbs.tile([EMB, 8], bf16, name=f"dd{u}",
                                     tag=f"dd{u}")
                    nc.gpsimd.tensor_sub(dd[u][:], hp[u], nn[u][:])
                for u in range(2):
                    zd[u] = sbs.tile([EMB, 8], bf16, name=f"zd{u}",
                                     tag=f"zd{u}")
                    nc.gpsimd.tensor_mul(zd[u][:], gates[u][:, 8:16],
                                         dd[u][:])
                for u in range(2):
                    nc.vector.tensor_add(
                        outT[:, 16 * t + 8 * u:16 * t + 8 * u + 8],
                        nn[u][:], zd[u][:])
            gru_scope.__exit__(None, None, None)
            # heads emitted after the loop: lower scheduler priority, so the
            # recurrence chain never waits behind head matmuls
            with nc.named_scope("heads"):
                for j in range(12):
                    head_chunk(j, 16)
                head_chunk(12, 8)  # last 64 rows (t in [192,200))

            if debug:
                nc.sync.dma_start(dbg["outT"][:], outT[:])

    nc.compile()
    return nc


def _host_prep(inputs):
    """Build the 8 per-core input maps from the full problem inputs."""
    from concourse import mybir
    f = np.float32
    bf = mybir.dt.np(mybir.dt.bfloat16)
    x = inputs["x"].astype(f, copy=False)
    ques = inputs["ques"].astype(f, copy=False)

    def T(a, dt=None):
        return np.ascontiguousarray(
            np.asarray(a).T.astype(dt or f, copy=False))

    # layer-1 GCN activations, computed on host (tiny)
    z1 = {"hg": ques @ inputs["hg_W1"] + inputs["hg_b1"],
          "g1": ques @ inputs["g1_W1"] + inputs["g1_b1"],
          "g2": ques @ inputs["g2_W1"] + inputs["g2_b1"]}
    graphs = {"hg": inputs["G"], "g1": inputs["adj_out"], "g2": inputs["adj_in"]}

    def kperm(a, w):
        # [NQ, w] -> [KC, NK*w] with arr2[p, k*w+j] = arr[k*KC+p, j]
        return np.ascontiguousarray(
            np.asarray(a).reshape(NK, KC, w).transpose(1, 0, 2)
            .reshape(KC, NK * w))

    shared = {
        "z1_hg": kperm(np.asarray(z1["hg"]).astype(bf), EMB),
        "z1_g1": kperm(np.asarray(z1["g1"]).astype(bf), EMB),
        "z1_g2": kperm(np.asarray(z1["g2"]).astype(bf), EMB),
        "w2_hg": np.ascontiguousarray(np.asarray(inputs["hg_W2"]).astype(bf)),
        "w2_g1": np.ascontiguousarray(np.asarray(inputs["g1_W2"]).astype(bf)),
        "w2_g2": np.ascontiguousarray(np.asarray(inputs["g2_W2"]).astype(bf)),
        "b2_hg": np.asarray(inputs["hg_b2"]).astype(bf).reshape(1, -1),
        "b2_g1": np.asarray(inputs["g1_b2"]).astype(bf).reshape(1, -1),
        "b2_g2": np.asarray(inputs["g2_b2"]).astype(bf).reshape(1, -1),
        "wihT1": T(inputs["r1_Wih"], bf),
        "wihT2": T(inputs["r2_Wih"], bf),
        "whhT1": T(inputs["r1_Whh"], bf),
        "whhT2": T(inputs["r2_Whh"], bf),
        "w1wT": T(inputs["w1_W"], bf),
        "w2wT": T(inputs["w2_W"], bf),
        "wb": np.asarray(inputs["w1_b"] + inputs["w2_b"], f).reshape(-1, 1),
        "fccwT": T(inputs["fcc_W"], bf),
        "fctwT": T(inputs["fct_W"], bf),
        "fcewT": T(inputs["fce_W"], bf),
    }
    for u, (ih, hh) in enumerate((("r1_bih", "r1_bhh"), ("r2_bih", "r2_bhh"))):
        bih = np.asarray(inputs[ih], f)
        bhh = np.asarray(inputs[hh], f)
        pb = np.zeros((EMB, 3), f)
        for g in range(3):
            pb[:, g] = bih[g * H:(g + 1) * H]
            if g < 2:  # r, z: fold bhh into the projection bias
                pb[:, g] += bhh[g * H:(g + 1) * H]
        shared[f"projb{u + 1}"] = pb
        shared[f"bhhn{u + 1}"] = \
            bhh[2 * H:3 * H].reshape(1, -1).astype(bf).copy()

    in_maps = []
    for c in range(NCORES):
        m = dict(shared)
        for g, arr in graphs.items():
            blk = np.asarray(arr)[c * SHARD:(c + 1) * SHARD, :]
            atc = np.zeros((NQ, SHARD_P), bf)
            atc[:, :SHARD] = blk.astype(f, copy=False).T.astype(bf)
            m[f"at_{g}"] = kperm(atc, SHARD_P)
        xc = x[c * BLOC:(c + 1) * BLOC]           # [8, 200, 5000]
        m["xt"] = kperm(
            xc.transpose(2, 1, 0).reshape(NQ, BLC).astype(bf), BLC)
        in_maps.append(m)
    return in_maps


def kernel(**inputs):
    global _BUILT, LAST
    from concourse import bass_utils
    if _BUILT is None:
        _BUILT = _build(debug=False)
    nc = _BUILT
    in_maps = _host_prep(inputs)
    res = bass_utils.run_bass_kernel_spmd(nc, in_maps,
                                          core_ids=list(range(NCORES)))
    LAST = res
    f = np.float32
    logit_c = np.empty((B, L, Q), f)
    logit_t = np.empty((B, L, Q), f)
    logit_e = np.empty((B, L, Q), f)
    for c in range(NCORES):
        r = res.results[c]
        logit_c[c * BLOC:(c + 1) * BLOC] = \
            r["out_c"].astype(f).transpose(1, 0, 2)
        logit_t[c * BLOC:(c + 1) * BLOC] = \
            r["out_t"].astype(f).transpose(1, 0, 2)
        logit_e[c * BLOC:(c + 1) * BLOC] = \
            r["out_e"].astype(f).transpose(1, 0, 2)
    for arr, bname in ((logit_c, "fcc_b"), (logit_t, "fct_b"),
                       (logit_e, "fce_b")):
        bias = np.asarray(inputs[bname], f)
        if np.any(bias):
            arr += bias
    return (logit_c, logit_t, logit_e)



# revision 69
# speedup vs baseline: 1.2439x; 1.0488x over previous
"""Trainium2 Bass kernel for the DKT (graph-based knowledge tracing) model.

Sharding across the 8 NeuronCores:
  - GCN phase: row-shard of the three [5000,5000] adjacency matmuls (625 rows
    per core), with AllGathers of the small [5000,EMB] intermediates.
  - x@ques / GRU / logit heads: data-parallel over batch (8 sequences/core).

All layouts are chosen so every matmul contracts over the partition dim with
contiguous DMA: the host passes A.T column-shards, x.T (time-major columns)
shards, host-computed layer-1 GCN activations, and transposed weights.
"""

import numpy as np

Q = 2500
NQ = 5000
EMB = 128
H = 128
B = 64
L = 200
NCORES = 8
SHARD = NQ // NCORES          # 625 adjacency rows per core
KC = 125                      # contraction chunk (partition dim)
NK = NQ // KC                 # 40 chunks
BLOC = B // NCORES            # 8 sequences per core
BLC = L * BLOC                # 1600 (bl index = t*8 + b, t-major)
SHARD_P = 640                 # shard padded to even halves (fp32r ISA rule)
NH = [(0, 320), (320, 320)]   # padded-shard column halves (>=256, even)
XNT = [(0, 512), (512, 512), (1024, 512), (1536, 64)]  # x-stage N tiles
HNT = [(0, 512), (512, 512), (1024, 512), (1536, 512), (2048, 452)]

_BUILT = None
LAST = None


def _build(debug=False):
    import concourse.bass as bass  # noqa: F401
    import concourse.tile as tile
    from concourse import bacc, mybir
    from concourse.masks import make_identity
    from contextlib import ExitStack

    f32 = mybir.dt.float32
    f32r = mybir.dt.float32r
    bf16 = mybir.dt.bfloat16
    AFT = mybir.ActivationFunctionType
    ALU = mybir.AluOpType

    nc = bacc.Bacc("TRN2", target_bir_lowering=False, debug=False,
                   num_devices=NCORES)

    def din(name, shape, dt=f32r):
        return nc.dram_tensor(name, shape, dt, kind="ExternalInput").ap()

    def dout(name, shape, dt=f32):
        return nc.dram_tensor(name, shape, dt, kind="ExternalOutput").ap()

    # --- inputs (per-core unless noted) ---
    # at/xt/z1 come host-permuted to k-chunk-contiguous per partition:
    # arr2[p, k*W + j] = arr[k*KC + p, j] so every DMA is a plain
    # contiguous [125, N] slice (SWDGE descriptor-gen cost was a
    # bottleneck for the strided "(k p) -> p k" views).
    at = {g: din(f"at_{g}", [KC, NK * SHARD_P], bf16)
          for g in ("hg", "g1", "g2")}
    xt = din("xt", [KC, NK * BLC], bf16)
    z1 = {g: din(f"z1_{g}", [KC, NK * EMB], bf16) for g in ("hg", "g1", "g2")}
    e2s = {"hg": EMB, "g1": EMB // 2, "g2": EMB // 2}
    w2 = {g: din(f"w2_{g}", [EMB, e2s[g]], bf16) for g in ("hg", "g1", "g2")}
    b2 = {g: din(f"b2_{g}", [1, e2s[g]], bf16) for g in ("hg", "g1", "g2")}
    wihT = [din("wihT1", [EMB, 3 * H], bf16), din("wihT2", [EMB, 3 * H], bf16)]
    whhT = [din("whhT1", [EMB, 3 * H], bf16), din("whhT2", [EMB, 3 * H], bf16)]
    projb = [din("projb1", [EMB, 3], f32), din("projb2", [EMB, 3], f32)]
    bhhn = [din("bhhn1", [1, EMB], bf16), din("bhhn2", [1, EMB], bf16)]
    w1wT = din("w1wT", [EMB, EMB], bf16)
    w2wT = din("w2wT", [EMB, EMB], bf16)
    wb = din("wb", [EMB, 1], f32)
    fccwT = din("fccwT", [EMB, Q], bf16)
    fctwT = din("fctwT", [EMB, Q], bf16)
    fcewT = din("fcewT", [2 * EMB, Q], bf16)

    out_c = dout("out_c", [L, BLOC, Q], bf16)
    out_t = dout("out_t", [L, BLOC, Q], bf16)
    out_e = dout("out_e", [L, BLOC, Q], bf16)
    dbg = {}
    if debug:
        dbg["qh"] = dout("dbg_qh", [NQ, EMB], bf16)
        dbg["qd"] = dout("dbg_qd", [NQ, EMB], bf16)
        dbg["xh"] = dout("dbg_xh", [EMB, BLC], bf16)
        dbg["xd"] = dout("dbg_xd", [EMB, BLC], bf16)
        dbg["xp"] = dout("dbg_xp", [EMB, L * 48], bf16)
        dbg["outT"] = dout("dbg_outT", [EMB, L * 16], bf16)

    with tile.TileContext(nc) as tc, ExitStack() as ctx:
        const = ctx.enter_context(tc.tile_pool(name="const", bufs=1))
        dram = ctx.enter_context(tc.tile_pool(name="dram", bufs=1, space="DRAM"))

        ident = const.tile([128, 128], f32, name="ident")
        make_identity(nc, ident[:])
        ones_f = const.tile([1, 128], f32, name="ones_f")
        nc.gpsimd.memset(ones_f[:], 1.0)
        ones = const.tile([1, 128], bf16, name="ones")
        nc.vector.tensor_copy(ones[:], ones_f[:])
        ident_r = const.tile([128, 128], f32r, name="ident_r")
        nc.vector.tensor_copy(ident_r[:], ident[:])

        # DRAM bounce buffers for the AllGathers
        zb = {"hg": dram.tile([SHARD, EMB], bf16, name="zb_hg"),
              "pr": dram.tile([SHARD, EMB], bf16, name="zb_pr")}
        zf = {"hg": dram.tile([NQ, EMB], bf16, name="zf_hg", addr_space="Shared"),
              "pr": dram.tile([NQ, EMB], bf16, name="zf_pr", addr_space="Shared")}
        qb = {"hg": dram.tile([SHARD, EMB], bf16, name="qb_hg"),
              "pr": dram.tile([SHARD, EMB], bf16, name="qb_pr")}
        qf = {"hg": dram.tile([NQ, EMB], bf16, name="qf_hg", addr_space="Shared"),
              "pr": dram.tile([NQ, EMB], bf16, name="qf_pr", addr_space="Shared")}
        RG = [list(range(NCORES))]

        def allgather(inb, outb):
            nc.gpsimd.collective_compute(
                "AllGather", ALU.bypass, replica_groups=RG,
                ins=[inb.opt()], outs=[outb.opt()])

        def rearr_kpe(ap, e):
            return ap.rearrange("(k p) e -> p k e", p=KC)

        # SWDGE (gpsimd) sprays one dma_start across all 16 SDMA engines;
        # a sync-queue (HWDGE) dma runs on a single engine (~36 GB/s) and
        # serializes. All bulk streams go through gpsimd.
        def big_dma(out, in_):
            nc.gpsimd.dma_start(out, in_)

        # ================= GCN phase =================
        # x-phase pools are allocated UP FRONT (disjoint SBUF/PSUM from the
        # GCN pools) so the x@qh pass can overlap the GCN tail instead of
        # inheriting write-after-read deps from address reuse.
        sbQ = ctx.enter_context(tc.tile_pool(name="sbQ", bufs=1))
        qh_sb = sbQ.tile([KC, NK * EMB], bf16, name="qh_sb")
        qd_sb = sbQ.tile([KC, NK * EMB], bf16, name="qd_sb")
        sbP = ctx.enter_context(tc.tile_pool(name="sbP", bufs=1))
        xp = sbP.tile([EMB, L * 48], bf16, name="xp")
        xp_v = xp.rearrange("p (t u g b) -> p t u g b", u=2, g=3, b=BLOC)
        sbX = ctx.enter_context(tc.tile_pool(name="sbX", bufs=1))
        xhT = sbX.tile([EMB, BLC], bf16, name="xhT")
        xdT = sbX.tile([EMB, BLC], bf16, name="xdT")
        xstream = ctx.enter_context(tc.tile_pool(name="xstream", bufs=2))
        with tc.tile_pool(name="sbG", bufs=1) as sbG, \
             tc.tile_pool(name="astream", bufs=2) as astream, \
             tc.tile_pool(name="psA", bufs=2, space="PSUM") as psA, \
             tc.tile_pool(name="psW", bufs=1, space="PSUM") as psW, \
             tc.tile_pool(name="psT", bufs=1, space="PSUM") as psT:

            z1sb, hT, w2sb, b2sb = {}, {}, {}, {}
            zstag, qstag, z2f = {}, {}, {}
            ahold = {}
            APIECE = [(0, 8), (8, 16), (24, 16)]   # k-chunk DMA pieces

            def gcn_stage1(g):
                e2 = e2s[g]
                z1sb[g] = sbG.tile([KC, NK * EMB], bf16, name=f"z1sb_{g}", tag="z1sb", bufs=1)
                big_dma(z1sb[g][:], z1[g][:])
                w2sb[g] = sbG.tile([EMB, e2], bf16, name=f"w2sb_{g}")
                nc.sync.dma_start(w2sb[g][:], w2[g][:])
                b2sb[g] = sbG.tile([1, e2], bf16, name=f"b2sb_{g}")
                nc.sync.dma_start(b2sb[g][:], b2[g][:])
                hT[g] = sbG.tile([EMB, SHARD_P], bf16, name=f"hT_{g}", tag="hT", bufs=2)

                # stream at_g ONCE into a held buffer; stage2a re-reads it
                # from SBUF (at traffic halved: DMA is the phase roofline)
                a_t = astream.tile([KC, NK * SHARD_P], bf16, name=f"a_{g}",
                                   tag="a")
                ahold[g] = a_t
                for k0, nk_ in APIECE:
                    big_dma(a_t[:, k0 * SHARD_P:(k0 + nk_) * SHARD_P],
                            at[g][:, k0 * SHARD_P:(k0 + nk_) * SHARD_P])
                ps = [psA.tile([EMB, 512], f32, name=f"ps1_{g}{i}", tag="psA")
                      for i in range(2)]
                for k in range(NK):
                    for i, (off, nh) in enumerate(NH):
                        nc.tensor.matmul(
                            ps[i][:, :nh],
                            z1sb[g][:, k * EMB:(k + 1) * EMB],
                            a_t[:, k * SHARD_P + off:k * SHARD_P + off + nh],
                            start=(k == 0), stop=(k == NK - 1))
                for i, (off, nh) in enumerate(NH):
                    nc.scalar.activation(hT[g][:, off:off + nh],
                                         ps[i][:EMB, :nh], AFT.Relu)

            def gcn_stage2w(g, grp, coloff):
                # Z2 = h @ W2 + b2 (natural layout, into the AG input staging)
                e2 = e2s[g]
                if grp not in zstag:
                    zstag[grp] = sbG.tile([KC, 5 * EMB], bf16,
                                          name=f"zstag_{grp}")
                for c in range(5):
                    ps = psW.tile([KC, EMB], f32, name="psW", tag="psW")
                    nc.tensor.matmul(ps[:, :e2], hT[g][:, c * KC:(c + 1) * KC],
                                     w2sb[g][:], start=True, stop=False)
                    nc.tensor.matmul(ps[:, :e2], ones[:, :KC], b2sb[g][:],
                                     start=False, stop=True)
                    nc.vector.tensor_copy(
                        zstag[grp][:, c * EMB + coloff: c * EMB + coloff + e2],
                        ps[:, :e2])

            def ag_z(grp):
                nc.sync.dma_start(
                    zb[grp].rearrange("(c p) e -> p c e", p=KC),
                    zstag[grp].rearrange("p (c e) -> p c e", c=5))
                allgather(zb[grp], zf[grp])
                z2f[grp] = sbG.tile([KC, NK * EMB], bf16, name=f"z2f_{grp}", tag="z2f", bufs=2)
                nc.gpsimd.dma_start(
                    z2f[grp].rearrange("p (k e) -> p k e", k=NK),
                    rearr_kpe(zf[grp], EMB))

            def gcn_stage2a(g, grp, coloff):
                e2 = e2s[g]
                o2T = sbG.tile([e2, SHARD_P], f32, name=f"o2T_{g}", tag="o2T", bufs=2)
                ps = [psA.tile([EMB, 512], f32, name=f"ps2_{g}{i}", tag="psA")
                      for i in range(2)]
                a_t = ahold[g]
                for k in range(NK):
                    for i, (off, nh) in enumerate(NH):
                        nc.tensor.matmul(
                            ps[i][:e2, :nh],
                            z2f[grp][:, k * EMB + coloff: k * EMB + coloff + e2],
                            a_t[:, k * SHARD_P + off:k * SHARD_P + off + nh],
                            start=(k == 0), stop=(k == NK - 1))
                for i, (off, nh) in enumerate(NH):
                    nc.vector.tensor_copy(o2T[:, off:off + nh], ps[i][:e2, :nh])
                # transpose to natural layout; stage for the output AllGather
                if grp not in qstag:
                    qstag[grp] = sbG.tile([KC, 5 * EMB], bf16,
                                          name=f"qstag_{grp}")
                # ques_d = concat([ques_in(g2), ques_out(g1)]): g2 -> cols
                # 0:64, g1 -> cols 64:128 of each block; hg -> full block.
                qoff = {"hg": 0, "g1": 64, "g2": 0}[g]
                for c in range(5):
                    pst = psT.tile([KC, EMB], f32, name="psT", tag="psT")
                    nc.tensor.transpose(pst[:, :e2],
                                        o2T[:, c * KC:(c + 1) * KC],
                                        ident[:e2, :e2])
                    nc.vector.tensor_copy(
                        qstag[grp][:, c * EMB + qoff: c * EMB + qoff + e2],
                        pst[:, :e2])

            def ag_q(grp):
                nc.sync.dma_start(
                    qb[grp].rearrange("(c p) e -> p c e", p=KC),
                    qstag[grp].rearrange("p (c e) -> p c e", c=5))
                allgather(qb[grp], qf[grp])

            with nc.named_scope("gcn"):
                gcn_stage1("hg")
                gcn_stage2w("hg", "hg", 0)
                ag_z("hg")
                gcn_stage1("g1")
                gcn_stage2a("hg", "hg", 0)
                ag_q("hg")
                nc.gpsimd.dma_start(qh_sb.rearrange("p (k e) -> p k e", k=NK),
                                    rearr_kpe(qf["hg"], EMB))
                gcn_stage2w("g1", "pr", 0)
                gcn_stage1("g2")
                gcn_stage2w("g2", "pr", 64)
                ag_z("pr")
                gcn_stage2a("g1", "pr", 0)
                gcn_stage2a("g2", "pr", 64)
                ag_q("pr")
                nc.gpsimd.dma_start(qd_sb.rearrange("p (k e) -> p k e", k=NK),
                                    rearr_kpe(qf["pr"], EMB))

        if debug:
            nc.sync.dma_start(dbg["qh"][:], qf["hg"][:])
            nc.sync.dma_start(dbg["qd"][:], qf["pr"][:])

        # ================= x @ ques phase =================
        XQ = 2                # k-chunks per stream piece
        psX_cm = tc.tile_pool(name="psX", bufs=1, space="PSUM")
        psX = psX_cm.__enter__()
        with nc.named_scope("xques"):
            psh = [psX.tile([EMB, 512], f32, name=f"psxh{i}", tag=f"psxh{i}")
                   for i in range(4)]
            psd = [psX.tile([EMB, 512], f32, name=f"psxd{i}", tag=f"psxd{i}")
                   for i in range(4)]
            for q in range(NK // XQ):
                xsb = xstream.tile([KC, XQ * BLC], bf16, name="xsb",
                                   tag="xsb")
                big_dma(xsb[:], xt[:, q * XQ * BLC:(q + 1) * XQ * BLC])
                for kk in range(XQ):
                    k = q * XQ + kk
                    for i, (off, nn_) in enumerate(XNT):
                        nc.tensor.matmul(psh[i][:, :nn_],
                                         qh_sb[:, k * EMB:(k + 1) * EMB],
                                         xsb[:, kk * BLC + off:kk * BLC + off + nn_],
                                         start=(k == 0), stop=(k == NK - 1))
                        nc.tensor.matmul(psd[i][:, :nn_],
                                         qd_sb[:, k * EMB:(k + 1) * EMB],
                                         xsb[:, kk * BLC + off:kk * BLC + off + nn_],
                                         start=(k == 0), stop=(k == NK - 1))
            for i, (off, nn_) in enumerate(XNT):
                nc.vector.tensor_copy(xhT[:, off:off + nn_], psh[i][:, :nn_])
                nc.vector.tensor_copy(xdT[:, off:off + nn_], psd[i][:, :nn_])

        if debug:
            nc.sync.dma_start(dbg["xh"][:], xhT[:])
            nc.sync.dma_start(dbg["xd"][:], xdT[:])
        psX_cm.__exit__(None, None, None)

        # ============ GRU input projections ============
        # xp column layout per step t: [xr1 xr2 xz1 xz2 xn1 xn2] (8 each)
        with tc.tile_pool(name="psP", bufs=3, space="PSUM") as psP, \
             tc.tile_pool(name="sbW", bufs=1) as sbW, \
             nc.named_scope("proj"):
            wih_sb, pb_sb = [], []
            for u in range(2):
                wt = sbW.tile([EMB, 3 * H], bf16, name=f"wihsb{u}")
                nc.sync.dma_start(wt[:], wihT[u][:])
                wih_sb.append(wt)
                pb = sbW.tile([EMB, 3], f32, name=f"pbsb{u}")
                nc.sync.dma_start(pb[:], projb[u][:])
                pb_sb.append(pb)
            # nt-major so the GRU's first steps unblock after the first tile
            for nt in range(4):
                for u in range(2):
                    src = xhT if u == 0 else xdT
                    for g in range(3):
                        ps = psP.tile([EMB, 400], f32, name="psP",
                                      tag="psP")
                        nc.tensor.matmul(
                            ps[:], wih_sb[u][:, g * H:(g + 1) * H],
                            src[:, nt * 400:(nt + 1) * 400],
                            start=True, stop=True)
                        nc.scalar.activation(
                            xp_v[:, nt * 50:(nt + 1) * 50, u, g, :],
                            ps.rearrange("p (t b) -> p t b", b=BLOC),
                            AFT.Identity, bias=pb_sb[u][:, g:g + 1])
        if debug:
            nc.sync.dma_start(dbg["xp"][:], xp[:])

        # ================= GRU + heads phase =================
        with tc.tile_pool(name="sbR", bufs=1) as sbR, \
             tc.tile_pool(name="sbh", bufs=2) as sbh, \
             tc.tile_pool(name="sbstep", bufs=4) as sbs, \
             tc.tile_pool(name="stg", bufs=2) as stg, \
             tc.tile_pool(name="psG", bufs=5, space="PSUM") as psG, \
             tc.tile_pool(name="psTh", bufs=1, space="PSUM") as psTh, \
             tc.tile_pool(name="psH", bufs=2, space="PSUM") as psH:
            whh_sb, bhhn_sb = [], []
            for u in range(2):
                wt = sbR.tile([EMB, 3 * H], bf16, name=f"whhsb{u}")
                nc.sync.dma_start(wt[:], whhT[u][:])
                whh_sb.append(wt)
                bt = sbR.tile([1, EMB], bf16, name=f"bhhnsb{u}")
                nc.sync.dma_start(bt[:], bhhn[u][:])
                bhhn_sb.append(bt)
            w1w_sb = sbR.tile([EMB, EMB], bf16, name="w1wsb")
            nc.sync.dma_start(w1w_sb[:], w1wT[:])
            w2w_sb = sbR.tile([EMB, EMB], bf16, name="w2wsb")
            nc.sync.dma_start(w2w_sb[:], w2wT[:])
            wb_sb = sbR.tile([EMB, 1], f32, name="wbsb")
            nc.sync.dma_start(wb_sb[:], wb[:])
            # head weights are not needed until ~the GRU: delay their DMA so
            # they don't steal HBM bandwidth from the GCN A-streams
            hw_sb = {}
            with tc.tile_wait_until(0.35):
                for nm, t_ in (("fcc", fccwT), ("fct", fctwT)):
                    w_ = sbR.tile([EMB, Q], bf16, name=f"{nm}wsb")
                    nc.sync.dma_start(w_[:], t_[:])
                    hw_sb[nm] = w_
                fce0 = sbR.tile([EMB, Q], bf16, name="fce0sb")
                nc.sync.dma_start(fce0[:], fcewT[0:EMB, :])
                fce1 = sbR.tile([EMB, Q], bf16, name="fce1sb")
                nc.sync.dma_start(fce1[:], fcewT[EMB:2 * EMB, :])

            outT = sbR.tile([EMB, L * 16], bf16, name="outT")
            outT_v = outT.rearrange("p (t u b) -> p t u b", u=2, b=BLOC)
            zero16_f = sbR.tile([EMB, 16], f32, name="zero16_f")
            nc.gpsimd.memset(zero16_f[:], 0.0)
            zero16 = sbR.tile([EMB, 16], bf16, name="zero16")
            nc.vector.tensor_copy(zero16[:], zero16_f[:])
            stag = {nm: stg.tile([128, Q], bf16, name=f"stag_{nm}")
                    for nm in ("c", "t", "e")}
            out_flat = {"c": out_c.rearrange("l b q -> (l b) q"),
                        "t": out_t.rearrange("l b q -> (l b) q"),
                        "e": out_e.rearrange("l b q -> (l b) q")}

            def head_chunk(j, nt16):
                rows = nt16 * BLOC
                lh = sbh.tile([EMB, 128], bf16, name="lh", tag="lh")
                ld = sbh.tile([EMB, 128], bf16, name="ld", tag="ld")
                nc.vector.tensor_copy(
                    lh[:, :rows].rearrange("p (t b) -> p t b", b=BLOC),
                    outT_v[:, 16 * j:16 * j + nt16, 0, :])
                nc.vector.tensor_copy(
                    ld[:, :rows].rearrange("p (t b) -> p t b", b=BLOC),
                    outT_v[:, 16 * j:16 * j + nt16, 1, :])
                pst = psTh.tile([EMB, 128], f32, name="pstheta", tag="pstheta")
                nc.tensor.matmul(pst[:, :rows], w1w_sb[:], lh[:, :rows],
                                 start=True, stop=False)
                nc.tensor.matmul(pst[:, :rows], w2w_sb[:], ld[:, :rows],
                                 start=False, stop=True)
                theta = sbh.tile([EMB, 128], bf16, name="theta", tag="theta")
                nc.scalar.activation(theta[:, :rows], pst[:, :rows],
                                     AFT.Sigmoid, bias=wb_sb[:])
                omt = sbh.tile([EMB, 128], bf16, name="omt", tag="omt")
                nc.scalar.activation(omt[:, :rows], theta[:, :rows],
                                     AFT.Identity, scale=-1.0, bias=1.0)
                od = sbh.tile([EMB, 128], bf16, name="od", tag="od")
                nc.vector.tensor_mul(od[:, :rows], theta[:, :rows],
                                     ld[:, :rows])
                oh = sbh.tile([EMB, 128], bf16, name="oh", tag="oh")
                nc.vector.tensor_mul(oh[:, :rows], omt[:, :rows],
                                     lh[:, :rows])
                for noff, nsz in HNT:
                    psc = psH.tile([128, 512], f32, name="psc", tag="psh")
                    nc.tensor.matmul(psc[:rows, :nsz], lh[:, :rows],
                                     hw_sb["fcc"][:, noff:noff + nsz],
                                     start=True, stop=True)
                    nc.scalar.activation(
                        stag["c"][:rows, noff:noff + nsz], psc[:rows, :nsz],
                        AFT.Identity)
                    psc = psH.tile([128, 512], f32, name="psc2", tag="psh")
                    nc.tensor.matmul(psc[:rows, :nsz], ld[:, :rows],
                                     hw_sb["fct"][:, noff:noff + nsz],
                                     start=True, stop=True)
                    nc.scalar.activation(
                        stag["t"][:rows, noff:noff + nsz], psc[:rows, :nsz],
                        AFT.Identity)
                    psc = psH.tile([128, 512], f32, name="psc3", tag="psh")
                    nc.tensor.matmul(psc[:rows, :nsz], od[:, :rows],
                                     fce0[:, noff:noff + nsz],
                                     start=True, stop=False)
                    nc.tensor.matmul(psc[:rows, :nsz], oh[:, :rows],
                                     fce1[:, noff:noff + nsz],
                                     start=False, stop=True)
                    nc.vector.tensor_copy(
                        stag["e"][:rows, noff:noff + nsz], psc[:rows, :nsz])
                for nm in ("c", "t", "e"):
                    nc.sync.dma_start(out_flat[nm][128 * j:128 * j + rows, :],
                                      stag[nm][:rows, :])

            gru_scope = nc.named_scope("gru")
            gru_scope.__enter__()
            # the two GRUs' chains are emitted op-by-op interleaved: the
            # static per-engine queue order then matches dataflow order, so
            # neither chain head-of-line-blocks the other.
            for t in range(L):
                hp, xb, psrz, psn = {}, {}, {}, {}
                psg = psG.tile([EMB, 48], f32, name="psg", tag="psg")
                for u in range(2):
                    hp[u] = (outT[:, 16 * (t - 1) + 8 * u:
                                  16 * (t - 1) + 8 * u + 8]
                             if t > 0 else zero16[:, 8 * u:8 * u + 8])
                    xb[u] = 48 * t + 24 * u
                    psrz[u] = psg[:, 24 * u:24 * u + 16]
                    psn[u] = psg[:, 24 * u + 16:24 * u + 24]
                    # bias fold is h-independent: keep PE busy at step start
                    nc.tensor.matmul(psn[u], bhhn_sb[u][:], ones[:, 0:8],
                                     start=True, stop=False)
                for u in range(2):
                    nc.tensor.matmul(psrz[u][:, 0:8], whh_sb[u][:, 0:H],
                                     hp[u], start=True, stop=True)
                    nc.tensor.matmul(psrz[u][:, 8:16], whh_sb[u][:, H:2 * H],
                                     hp[u], start=True, stop=True)
                    nc.tensor.matmul(psn[u], whh_sb[u][:, 2 * H:3 * H], hp[u],
                                     start=False, stop=True)
                rzp, gates, rn, npre, nn, dd, zd = ({} for _ in range(7))
                for u in range(2):
                    rzp[u] = sbs.tile([EMB, 16], f32r, name=f"rzp{u}",
                                      tag=f"rzp{u}")
                    nc.vector.tensor_add(rzp[u][:], psrz[u].bitcast(f32r),
                                         xp[:, xb[u]:xb[u] + 16])
                for u in range(2):
                    gates[u] = sbs.tile([EMB, 16], bf16, name=f"gates{u}",
                                        tag=f"gates{u}")
                    nc.scalar.activation(gates[u][:], rzp[u][:], AFT.Sigmoid)
                for u in range(2):
                    rn[u] = sbs.tile([EMB, 8], f32r, name=f"rn{u}",
                                     tag=f"rn{u}")
                    nc.vector.tensor_mul(rn[u][:], psn[u].bitcast(f32r),
                                         gates[u][:, 0:8])
                for u in range(2):
                    npre[u] = sbs.tile([EMB, 8], f32r, name=f"npre{u}",
                                       tag=f"npre{u}")
                    nc.vector.tensor_add(npre[u][:], rn[u][:],
                                         xp[:, xb[u] + 16:xb[u] + 24])
                for u in range(2):
                    nn[u] = sbs.tile([EMB, 8], bf16, name=f"nn{u}",
                                     tag=f"nn{u}")
                    nc.scalar.activation(nn[u][:], npre[u][:], AFT.Tanh)
                for u in range(2):
                    dd[u] = sbs.tile([EMB, 8], bf16, name=f"dd{u}",
                                     tag=f"dd{u}")
                    nc.vector.tensor_sub(dd[u][:], hp[u], nn[u][:])
                for u in range(2):
                    zd[u] = sbs.tile([EMB, 8], bf16, name=f"zd{u}",
                                     tag=f"zd{u}")
                    nc.vector.tensor_mul(zd[u][:], gates[u][:, 8:16],
                                         dd[u][:])
                for u in range(2):
                    nc.vector.tensor_add(
                        outT[:, 16 * t + 8 * u:16 * t + 8 * u + 8],
                        nn[u][:], zd[u][:])
            gru_scope.__exit__(None, None, None)
            # heads emitted after the loop: lower scheduler priority, so the
            # recurrence chain never waits behind head matmuls
            with nc.named_scope("heads"):
                for j in range(12):
                    head_chunk(j, 16)
                head_chunk(12, 8)  # last 64 rows (t in [192,200))

            if debug:
                nc.sync.dma_start(dbg["outT"][:], outT[:])

    nc.compile()
    return nc


def _host_prep(inputs):
    """Build the 8 per-core input maps from the full problem inputs."""
    from concourse import mybir
    f = np.float32
    bf = mybir.dt.np(mybir.dt.bfloat16)
    x = inputs["x"].astype(f, copy=False)
    ques = inputs["ques"].astype(f, copy=False)

    def T(a, dt=None):
        return np.ascontiguousarray(
            np.asarray(a).T.astype(dt or f, copy=False))

    # layer-1 GCN activations, computed on host (tiny)
    z1 = {"hg": ques @ inputs["hg_W1"] + inputs["hg_b1"],
          "g1": ques @ inputs["g1_W1"] + inputs["g1_b1"],
          "g2": ques @ inputs["g2_W1"] + inputs["g2_b1"]}
    graphs = {"hg": inputs["G"], "g1": inputs["adj_out"], "g2": inputs["adj_in"]}

    def kperm(a, w):
        # [NQ, w] -> [KC, NK*w] with arr2[p, k*w+j] = arr[k*KC+p, j]
        return np.ascontiguousarray(
            np.asarray(a).reshape(NK, KC, w).transpose(1, 0, 2)
            .reshape(KC, NK * w))

    shared = {
        "z1_hg": kperm(np.asarray(z1["hg"]).astype(bf), EMB),
        "z1_g1": kperm(np.asarray(z1["g1"]).astype(bf), EMB),
        "z1_g2": kperm(np.asarray(z1["g2"]).astype(bf), EMB),
        "w2_hg": np.ascontiguousarray(np.asarray(inputs["hg_W2"]).astype(bf)),
        "w2_g1": np.ascontiguousarray(np.asarray(inputs["g1_W2"]).astype(bf)),
        "w2_g2": np.ascontiguousarray(np.asarray(inputs["g2_W2"]).astype(bf)),
        "b2_hg": np.asarray(inputs["hg_b2"]).astype(bf).reshape(1, -1),
        "b2_g1": np.asarray(inputs["g1_b2"]).astype(bf).reshape(1, -1),
        "b2_g2": np.asarray(inputs["g2_b2"]).astype(bf).reshape(1, -1),
        "wihT1": T(inputs["r1_Wih"], bf),
        "wihT2": T(inputs["r2_Wih"], bf),
        "whhT1": T(inputs["r1_Whh"], bf),
        "whhT2": T(inputs["r2_Whh"], bf),
        "w1wT": T(inputs["w1_W"], bf),
        "w2wT": T(inputs["w2_W"], bf),
        "wb": np.asarray(inputs["w1_b"] + inputs["w2_b"], f).reshape(-1, 1),
        "fccwT": T(inputs["fcc_W"], bf),
        "fctwT": T(inputs["fct_W"], bf),
        "fcewT": T(inputs["fce_W"], bf),
    }
    for u, (ih, hh) in enumerate((("r1_bih", "r1_bhh"), ("r2_bih", "r2_bhh"))):
        bih = np.asarray(inputs[ih], f)
        bhh = np.asarray(inputs[hh], f)
        pb = np.zeros((EMB, 3), f)
        for g in range(3):
            pb[:, g] = bih[g * H:(g + 1) * H]
            if g < 2:  # r, z: fold bhh into the projection bias
                pb[:, g] += bhh[g * H:(g + 1) * H]
        shared[f"projb{u + 1}"] = pb
        shared[f"bhhn{u + 1}"] = \
            bhh[2 * H:3 * H].reshape(1, -1).astype(bf).copy()

    in_maps = []
    for c in range(NCORES):
        m = dict(shared)
        for g, arr in graphs.items():
            blk = np.asarray(arr)[c * SHARD:(c + 1) * SHARD, :]
            atc = np.zeros((NQ, SHARD_P), bf)
            atc[:, :SHARD] = blk.astype(f, copy=False).T.astype(bf)
            m[f"at_{g}"] = kperm(atc, SHARD_P)
        xc = x[c * BLOC:(c + 1) * BLOC]           # [8, 200, 5000]
        m["xt"] = kperm(
            xc.transpose(2, 1, 0).reshape(NQ, BLC).astype(bf), BLC)
        in_maps.append(m)
    return in_maps


def kernel(**inputs):
    global _BUILT, LAST
    from concourse import bass_utils
    if _BUILT is None:
        _BUILT = _build(debug=False)
    nc = _BUILT
    in_maps = _host_prep(inputs)
    res = bass_utils.run_bass_kernel_spmd(nc, in_maps,
                                          core_ids=list(range(NCORES)))
    LAST = res
    f = np.float32
    logit_c = np.empty((B, L, Q), f)
    logit_t = np.empty((B, L, Q), f)
    logit_e = np.empty((B, L, Q), f)
    for c in range(NCORES):
        r = res.results[c]
        logit_c[c * BLOC:(c + 1) * BLOC] = \
            r["out_c"].astype(f).transpose(1, 0, 2)
        logit_t[c * BLOC:(c + 1) * BLOC] = \
            r["out_t"].astype(f).transpose(1, 0, 2)
        logit_e[c * BLOC:(c + 1) * BLOC] = \
            r["out_e"].astype(f).transpose(1, 0, 2)
    for arr, bname in ((logit_c, "fcc_b"), (logit_t, "fct_b"),
                       (logit_e, "fce_b")):
        bias = np.asarray(inputs[bname], f)
        if np.any(bias):
            arr += bias
    return (logit_c, logit_t, logit_e)



# revision 70
# speedup vs baseline: 1.3006x; 1.0456x over previous
"""Trainium2 Bass kernel for the DKT (graph-based knowledge tracing) model.

Sharding across the 8 NeuronCores:
  - GCN phase: row-shard of the three [5000,5000] adjacency matmuls (625 rows
    per core), with AllGathers of the small [5000,EMB] intermediates.
  - x@ques / GRU / logit heads: data-parallel over batch (8 sequences/core).

All layouts are chosen so every matmul contracts over the partition dim with
contiguous DMA: the host passes A.T column-shards, x.T (time-major columns)
shards, host-computed layer-1 GCN activations, and transposed weights.
"""

import numpy as np

Q = 2500
NQ = 5000
EMB = 128
H = 128
B = 64
L = 200
NCORES = 8
SHARD = NQ // NCORES          # 625 adjacency rows per core
KC = 125                      # contraction chunk (partition dim)
NK = NQ // KC                 # 40 chunks
BLOC = B // NCORES            # 8 sequences per core
BLC = L * BLOC                # 1600 (bl index = t*8 + b, t-major)
SHARD_P = 640                 # shard padded to even halves (fp32r ISA rule)
NH = [(0, 320), (320, 320)]   # padded-shard column halves (>=256, even)
XNT = [(0, 512), (512, 512), (1024, 512), (1536, 64)]  # x-stage N tiles
HNT = [(0, 512), (512, 512), (1024, 512), (1536, 512), (2048, 452)]

_BUILT = None
LAST = None


def _build(debug=False):
    import concourse.bass as bass  # noqa: F401
    import concourse.tile as tile
    from concourse import bacc, mybir
    from concourse.masks import make_identity
    from contextlib import ExitStack

    f32 = mybir.dt.float32
    f32r = mybir.dt.float32r
    bf16 = mybir.dt.bfloat16
    AFT = mybir.ActivationFunctionType
    ALU = mybir.AluOpType

    nc = bacc.Bacc("TRN2", target_bir_lowering=False, debug=False,
                   num_devices=NCORES)

    def din(name, shape, dt=f32r):
        return nc.dram_tensor(name, shape, dt, kind="ExternalInput").ap()

    def dout(name, shape, dt=f32):
        return nc.dram_tensor(name, shape, dt, kind="ExternalOutput").ap()

    # --- inputs (per-core unless noted) ---
    # at/xt/z1 come host-permuted to k-chunk-contiguous per partition:
    # arr2[p, k*W + j] = arr[k*KC + p, j] so every DMA is a plain
    # contiguous [125, N] slice (SWDGE descriptor-gen cost was a
    # bottleneck for the strided "(k p) -> p k" views).
    at = {g: din(f"at_{g}", [KC, NK * SHARD_P], bf16)
          for g in ("hg", "g1", "g2")}
    xt = din("xt", [KC, NK * BLC], bf16)
    z1 = {g: din(f"z1_{g}", [KC, NK * EMB], bf16) for g in ("hg", "g1", "g2")}
    e2s = {"hg": EMB, "g1": EMB // 2, "g2": EMB // 2}
    w2 = {g: din(f"w2_{g}", [EMB, e2s[g]], bf16) for g in ("hg", "g1", "g2")}
    b2 = {g: din(f"b2_{g}", [1, e2s[g]], bf16) for g in ("hg", "g1", "g2")}
    wihT = [din("wihT1", [EMB, 3 * H], bf16), din("wihT2", [EMB, 3 * H], bf16)]
    whhT = [din("whhT1", [EMB, 3 * H], bf16), din("whhT2", [EMB, 3 * H], bf16)]
    projb = [din("projb1", [EMB, 3], f32), din("projb2", [EMB, 3], f32)]
    bhhn = [din("bhhn1", [1, EMB], bf16), din("bhhn2", [1, EMB], bf16)]
    w1wT = din("w1wT", [EMB, EMB], bf16)
    w2wT = din("w2wT", [EMB, EMB], bf16)
    wb = din("wb", [EMB, 1], f32)
    fccwT = din("fccwT", [EMB, Q], bf16)
    fctwT = din("fctwT", [EMB, Q], bf16)
    fcewT = din("fcewT", [2 * EMB, Q], bf16)

    out_c = dout("out_c", [L, BLOC, Q], bf16)
    out_t = dout("out_t", [L, BLOC, Q], bf16)
    out_e = dout("out_e", [L, BLOC, Q], bf16)
    dbg = {}
    if debug:
        dbg["qh"] = dout("dbg_qh", [NQ, EMB], bf16)
        dbg["qd"] = dout("dbg_qd", [NQ, EMB], bf16)
        dbg["xh"] = dout("dbg_xh", [EMB, BLC], bf16)
        dbg["xd"] = dout("dbg_xd", [EMB, BLC], bf16)
        dbg["xp"] = dout("dbg_xp", [EMB, L * 48], bf16)
        dbg["outT"] = dout("dbg_outT", [EMB, L * 16], bf16)

    with tile.TileContext(nc) as tc, ExitStack() as ctx:
        const = ctx.enter_context(tc.tile_pool(name="const", bufs=1))
        dram = ctx.enter_context(tc.tile_pool(name="dram", bufs=1, space="DRAM"))

        ident = const.tile([128, 128], f32, name="ident")
        make_identity(nc, ident[:])
        ones_f = const.tile([1, 128], f32, name="ones_f")
        nc.gpsimd.memset(ones_f[:], 1.0)
        ones = const.tile([1, 128], bf16, name="ones")
        nc.vector.tensor_copy(ones[:], ones_f[:])
        ident_r = const.tile([128, 128], f32r, name="ident_r")
        nc.vector.tensor_copy(ident_r[:], ident[:])

        # DRAM bounce buffers for the AllGathers
        zb = {"hg": dram.tile([SHARD, EMB], bf16, name="zb_hg"),
              "pr": dram.tile([SHARD, EMB], bf16, name="zb_pr")}
        zf = {"hg": dram.tile([NQ, EMB], bf16, name="zf_hg", addr_space="Shared"),
              "pr": dram.tile([NQ, EMB], bf16, name="zf_pr", addr_space="Shared")}
        qb = {"hg": dram.tile([SHARD, EMB], bf16, name="qb_hg"),
              "pr": dram.tile([SHARD, EMB], bf16, name="qb_pr")}
        qf = {"hg": dram.tile([NQ, EMB], bf16, name="qf_hg", addr_space="Shared"),
              "pr": dram.tile([NQ, EMB], bf16, name="qf_pr", addr_space="Shared")}
        RG = [list(range(NCORES))]

        def allgather(inb, outb):
            nc.gpsimd.collective_compute(
                "AllGather", ALU.bypass, replica_groups=RG,
                ins=[inb.opt()], outs=[outb.opt()])

        def rearr_kpe(ap, e):
            return ap.rearrange("(k p) e -> p k e", p=KC)

        # SWDGE (gpsimd) sprays one dma_start across all 16 SDMA engines;
        # a sync-queue (HWDGE) dma runs on a single engine (~36 GB/s) and
        # serializes. All bulk streams go through gpsimd.
        def big_dma(out, in_):
            nc.gpsimd.dma_start(out, in_)

        # ================= GCN phase =================
        # x-phase pools are allocated UP FRONT (disjoint SBUF/PSUM from the
        # GCN pools) so the x@qh pass can overlap the GCN tail instead of
        # inheriting write-after-read deps from address reuse.
        sbQ = ctx.enter_context(tc.tile_pool(name="sbQ", bufs=1))
        qh_sb = sbQ.tile([KC, NK * EMB], bf16, name="qh_sb")
        qd_sb = sbQ.tile([KC, NK * EMB], bf16, name="qd_sb")
        sbP = ctx.enter_context(tc.tile_pool(name="sbP", bufs=1))
        xp = sbP.tile([EMB, L * 48], bf16, name="xp")
        xp_v = xp.rearrange("p (t u g b) -> p t u g b", u=2, g=3, b=BLOC)
        sbX = ctx.enter_context(tc.tile_pool(name="sbX", bufs=1))
        xhT = sbX.tile([EMB, BLC], bf16, name="xhT")
        xdT = sbX.tile([EMB, BLC], bf16, name="xdT")
        xstream = ctx.enter_context(tc.tile_pool(name="xstream", bufs=2))
        with tc.tile_pool(name="sbG", bufs=1) as sbG, \
             tc.tile_pool(name="astream", bufs=2) as astream, \
             tc.tile_pool(name="psA", bufs=2, space="PSUM") as psA, \
             tc.tile_pool(name="psW", bufs=1, space="PSUM") as psW, \
             tc.tile_pool(name="psT", bufs=1, space="PSUM") as psT:

            z1sb, hT, w2sb, b2sb = {}, {}, {}, {}
            zstag, qstag, z2f = {}, {}, {}
            ahold = {}
            APIECE = [(0, 8), (8, 16), (24, 16)]   # k-chunk DMA pieces

            def gcn_stage1(g):
                e2 = e2s[g]
                z1sb[g] = sbG.tile([KC, NK * EMB], bf16, name=f"z1sb_{g}", tag="z1sb", bufs=1)
                big_dma(z1sb[g][:], z1[g][:])
                w2sb[g] = sbG.tile([EMB, e2], bf16, name=f"w2sb_{g}")
                nc.sync.dma_start(w2sb[g][:], w2[g][:])
                b2sb[g] = sbG.tile([1, e2], bf16, name=f"b2sb_{g}")
                nc.sync.dma_start(b2sb[g][:], b2[g][:])
                hT[g] = sbG.tile([EMB, SHARD_P], bf16, name=f"hT_{g}", tag="hT", bufs=2)

                # stream at_g ONCE into a held buffer; stage2a re-reads it
                # from SBUF (at traffic halved: DMA is the phase roofline)
                a_t = astream.tile([KC, NK * SHARD_P], bf16, name=f"a_{g}",
                                   tag="a")
                ahold[g] = a_t
                for k0, nk_ in APIECE:
                    big_dma(a_t[:, k0 * SHARD_P:(k0 + nk_) * SHARD_P],
                            at[g][:, k0 * SHARD_P:(k0 + nk_) * SHARD_P])
                ps = [psA.tile([EMB, 512], f32, name=f"ps1_{g}{i}", tag="psA")
                      for i in range(2)]
                for k in range(NK):
                    for i, (off, nh) in enumerate(NH):
                        nc.tensor.matmul(
                            ps[i][:, :nh],
                            z1sb[g][:, k * EMB:(k + 1) * EMB],
                            a_t[:, k * SHARD_P + off:k * SHARD_P + off + nh],
                            start=(k == 0), stop=(k == NK - 1))
                for i, (off, nh) in enumerate(NH):
                    nc.scalar.activation(hT[g][:, off:off + nh],
                                         ps[i][:EMB, :nh], AFT.Relu)

            def gcn_stage2w(g, grp, coloff):
                # Z2 = h @ W2 + b2 (natural layout, into the AG input staging)
                e2 = e2s[g]
                if grp not in zstag:
                    zstag[grp] = sbG.tile([KC, 5 * EMB], bf16,
                                          name=f"zstag_{grp}")
                for c in range(5):
                    ps = psW.tile([KC, EMB], f32, name="psW", tag="psW")
                    nc.tensor.matmul(ps[:, :e2], hT[g][:, c * KC:(c + 1) * KC],
                                     w2sb[g][:], start=True, stop=False)
                    nc.tensor.matmul(ps[:, :e2], ones[:, :KC], b2sb[g][:],
                                     start=False, stop=True)
                    nc.vector.tensor_copy(
                        zstag[grp][:, c * EMB + coloff: c * EMB + coloff + e2],
                        ps[:, :e2])

            def ag_z(grp):
                nc.sync.dma_start(
                    zb[grp].rearrange("(c p) e -> p c e", p=KC),
                    zstag[grp].rearrange("p (c e) -> p c e", c=5))
                allgather(zb[grp], zf[grp])
                z2f[grp] = sbG.tile([KC, NK * EMB], bf16, name=f"z2f_{grp}", tag="z2f", bufs=2)
                nc.gpsimd.dma_start(
                    z2f[grp].rearrange("p (k e) -> p k e", k=NK),
                    rearr_kpe(zf[grp], EMB))

            def gcn_stage2a(g, grp, coloff):
                e2 = e2s[g]
                o2T = sbG.tile([e2, SHARD_P], f32, name=f"o2T_{g}", tag="o2T", bufs=2)
                ps = [psA.tile([EMB, 512], f32, name=f"ps2_{g}{i}", tag="psA")
                      for i in range(2)]
                a_t = ahold[g]
                for k in range(NK):
                    for i, (off, nh) in enumerate(NH):
                        nc.tensor.matmul(
                            ps[i][:e2, :nh],
                            z2f[grp][:, k * EMB + coloff: k * EMB + coloff + e2],
                            a_t[:, k * SHARD_P + off:k * SHARD_P + off + nh],
                            start=(k == 0), stop=(k == NK - 1))
                for i, (off, nh) in enumerate(NH):
                    nc.vector.tensor_copy(o2T[:, off:off + nh], ps[i][:e2, :nh])
                # transpose to natural layout; stage for the output AllGather
                if grp not in qstag:
                    qstag[grp] = sbG.tile([KC, 5 * EMB], bf16,
                                          name=f"qstag_{grp}")
                # ques_d = concat([ques_in(g2), ques_out(g1)]): g2 -> cols
                # 0:64, g1 -> cols 64:128 of each block; hg -> full block.
                qoff = {"hg": 0, "g1": 64, "g2": 0}[g]
                for c in range(5):
                    pst = psT.tile([KC, EMB], f32, name="psT", tag="psT")
                    nc.tensor.transpose(pst[:, :e2],
                                        o2T[:, c * KC:(c + 1) * KC],
                                        ident[:e2, :e2])
                    nc.vector.tensor_copy(
                        qstag[grp][:, c * EMB + qoff: c * EMB + qoff + e2],
                        pst[:, :e2])

            def ag_q(grp):
                nc.sync.dma_start(
                    qb[grp].rearrange("(c p) e -> p c e", p=KC),
                    qstag[grp].rearrange("p (c e) -> p c e", c=5))
                allgather(qb[grp], qf[grp])

            with nc.named_scope("gcn"):
                gcn_stage1("hg")
                gcn_stage2w("hg", "hg", 0)
                ag_z("hg")
                gcn_stage1("g1")
                gcn_stage2a("hg", "hg", 0)
                ag_q("hg")
                nc.gpsimd.dma_start(qh_sb.rearrange("p (k e) -> p k e", k=NK),
                                    rearr_kpe(qf["hg"], EMB))
                gcn_stage2w("g1", "pr", 0)
                gcn_stage1("g2")
                gcn_stage2w("g2", "pr", 64)
                ag_z("pr")
                gcn_stage2a("g1", "pr", 0)
                gcn_stage2a("g2", "pr", 64)
                ag_q("pr")
                nc.gpsimd.dma_start(qd_sb.rearrange("p (k e) -> p k e", k=NK),
                                    rearr_kpe(qf["pr"], EMB))

        if debug:
            nc.sync.dma_start(dbg["qh"][:], qf["hg"][:])
            nc.sync.dma_start(dbg["qd"][:], qf["pr"][:])

        # ================= x @ ques phase =================
        XQ = 2                # k-chunks per stream piece
        psX_cm = tc.tile_pool(name="psX", bufs=1, space="PSUM")
        psX = psX_cm.__enter__()
        with nc.named_scope("xques"):
            psh = [psX.tile([EMB, 512], f32, name=f"psxh{i}", tag=f"psxh{i}")
                   for i in range(4)]
            psd = [psX.tile([EMB, 512], f32, name=f"psxd{i}", tag=f"psxd{i}")
                   for i in range(4)]
            for q in range(NK // XQ):
                xsb = xstream.tile([KC, XQ * BLC], bf16, name="xsb",
                                   tag="xsb")
                big_dma(xsb[:], xt[:, q * XQ * BLC:(q + 1) * XQ * BLC])
                for kk in range(XQ):
                    k = q * XQ + kk
                    for i, (off, nn_) in enumerate(XNT):
                        nc.tensor.matmul(psh[i][:, :nn_],
                                         qh_sb[:, k * EMB:(k + 1) * EMB],
                                         xsb[:, kk * BLC + off:kk * BLC + off + nn_],
                                         start=(k == 0), stop=(k == NK - 1))
                        nc.tensor.matmul(psd[i][:, :nn_],
                                         qd_sb[:, k * EMB:(k + 1) * EMB],
                                         xsb[:, kk * BLC + off:kk * BLC + off + nn_],
                                         start=(k == 0), stop=(k == NK - 1))
            for i, (off, nn_) in enumerate(XNT):
                nc.vector.tensor_copy(xhT[:, off:off + nn_], psh[i][:, :nn_])
                nc.vector.tensor_copy(xdT[:, off:off + nn_], psd[i][:, :nn_])

        if debug:
            nc.sync.dma_start(dbg["xh"][:], xhT[:])
            nc.sync.dma_start(dbg["xd"][:], xdT[:])
        psX_cm.__exit__(None, None, None)

        # ============ GRU input projections ============
        # xp column layout per step t: [xr1 xr2 xz1 xz2 xn1 xn2] (8 each)
        with tc.tile_pool(name="psP", bufs=3, space="PSUM") as psP, \
             tc.tile_pool(name="sbW", bufs=1) as sbW, \
             nc.named_scope("proj"):
            wih_sb, pb_sb = [], []
            for u in range(2):
                wt = sbW.tile([EMB, 3 * H], bf16, name=f"wihsb{u}")
                nc.sync.dma_start(wt[:], wihT[u][:])
                wih_sb.append(wt)
                pb = sbW.tile([EMB, 3], f32, name=f"pbsb{u}")
                nc.sync.dma_start(pb[:], projb[u][:])
                pb_sb.append(pb)
            # nt-major so the GRU's first steps unblock after the first tile
            for nt in range(4):
                for u in range(2):
                    src = xhT if u == 0 else xdT
                    for g in range(3):
                        ps = psP.tile([EMB, 400], f32, name="psP",
                                      tag="psP")
                        nc.tensor.matmul(
                            ps[:], wih_sb[u][:, g * H:(g + 1) * H],
                            src[:, nt * 400:(nt + 1) * 400],
                            start=True, stop=True)
                        nc.scalar.activation(
                            xp_v[:, nt * 50:(nt + 1) * 50, u, g, :],
                            ps.rearrange("p (t b) -> p t b", b=BLOC),
                            AFT.Identity, bias=pb_sb[u][:, g:g + 1])
        if debug:
            nc.sync.dma_start(dbg["xp"][:], xp[:])

        # ================= GRU + heads phase =================
        with tc.tile_pool(name="sbR", bufs=1) as sbR, \
             tc.tile_pool(name="sbh", bufs=2) as sbh, \
             tc.tile_pool(name="sbstep", bufs=4) as sbs, \
             tc.tile_pool(name="stg", bufs=2) as stg, \
             tc.tile_pool(name="psG", bufs=5, space="PSUM") as psG, \
             tc.tile_pool(name="psTh", bufs=1, space="PSUM") as psTh, \
             tc.tile_pool(name="psH", bufs=2, space="PSUM") as psH:
            whh_sb, bhhn_sb = [], []
            for u in range(2):
                wt = sbR.tile([EMB, 3 * H], bf16, name=f"whhsb{u}")
                nc.sync.dma_start(wt[:], whhT[u][:])
                whh_sb.append(wt)
                bt = sbR.tile([1, EMB], bf16, name=f"bhhnsb{u}")
                nc.sync.dma_start(bt[:], bhhn[u][:])
                bhhn_sb.append(bt)
            w1w_sb = sbR.tile([EMB, EMB], bf16, name="w1wsb")
            nc.sync.dma_start(w1w_sb[:], w1wT[:])
            w2w_sb = sbR.tile([EMB, EMB], bf16, name="w2wsb")
            nc.sync.dma_start(w2w_sb[:], w2wT[:])
            wb_sb = sbR.tile([EMB, 1], f32, name="wbsb")
            nc.sync.dma_start(wb_sb[:], wb[:])
            # head weights are not needed until ~the GRU: delay their DMA so
            # they don't steal HBM bandwidth from the GCN A-streams
            hw_sb = {}
            with tc.tile_wait_until(0.35):
                for nm, t_ in (("fcc", fccwT), ("fct", fctwT)):
                    w_ = sbR.tile([EMB, Q], bf16, name=f"{nm}wsb")
                    nc.sync.dma_start(w_[:], t_[:])
                    hw_sb[nm] = w_
                fce0 = sbR.tile([EMB, Q], bf16, name="fce0sb")
                nc.sync.dma_start(fce0[:], fcewT[0:EMB, :])
                fce1 = sbR.tile([EMB, Q], bf16, name="fce1sb")
                nc.sync.dma_start(fce1[:], fcewT[EMB:2 * EMB, :])

            outT = sbR.tile([EMB, L * 16], bf16, name="outT")
            outT_v = outT.rearrange("p (t u b) -> p t u b", u=2, b=BLOC)
            zero16_f = sbR.tile([EMB, 16], f32, name="zero16_f")
            nc.gpsimd.memset(zero16_f[:], 0.0)
            zero16 = sbR.tile([EMB, 16], bf16, name="zero16")
            nc.vector.tensor_copy(zero16[:], zero16_f[:])
            stag = {nm: stg.tile([128, Q], bf16, name=f"stag_{nm}")
                    for nm in ("c", "t", "e")}
            out_flat = {"c": out_c.rearrange("l b q -> (l b) q"),
                        "t": out_t.rearrange("l b q -> (l b) q"),
                        "e": out_e.rearrange("l b q -> (l b) q")}

            def head_chunk(j, nt16):
                rows = nt16 * BLOC
                lh = sbh.tile([EMB, 128], bf16, name="lh", tag="lh")
                ld = sbh.tile([EMB, 128], bf16, name="ld", tag="ld")
                nc.vector.tensor_copy(
                    lh[:, :rows].rearrange("p (t b) -> p t b", b=BLOC),
                    outT_v[:, 16 * j:16 * j + nt16, 0, :])
                nc.vector.tensor_copy(
                    ld[:, :rows].rearrange("p (t b) -> p t b", b=BLOC),
                    outT_v[:, 16 * j:16 * j + nt16, 1, :])
                pst = psTh.tile([EMB, 128], f32, name="pstheta", tag="pstheta")
                nc.tensor.matmul(pst[:, :rows], w1w_sb[:], lh[:, :rows],
                                 start=True, stop=False)
                nc.tensor.matmul(pst[:, :rows], w2w_sb[:], ld[:, :rows],
                                 start=False, stop=True)
                theta = sbh.tile([EMB, 128], bf16, name="theta", tag="theta")
                nc.scalar.activation(theta[:, :rows], pst[:, :rows],
                                     AFT.Sigmoid, bias=wb_sb[:])
                omt = sbh.tile([EMB, 128], bf16, name="omt", tag="omt")
                nc.scalar.activation(omt[:, :rows], theta[:, :rows],
                                     AFT.Identity, scale=-1.0, bias=1.0)
                od = sbh.tile([EMB, 128], bf16, name="od", tag="od")
                nc.vector.tensor_mul(od[:, :rows], theta[:, :rows],
                                     ld[:, :rows])
                oh = sbh.tile([EMB, 128], bf16, name="oh", tag="oh")
                nc.vector.tensor_mul(oh[:, :rows], omt[:, :rows],
                                     lh[:, :rows])
                for noff, nsz in HNT:
                    psc = psH.tile([128, 512], f32, name="psc", tag="psh")
                    nc.tensor.matmul(psc[:rows, :nsz], lh[:, :rows],
                                     hw_sb["fcc"][:, noff:noff + nsz],
                                     start=True, stop=True)
                    nc.scalar.activation(
                        stag["c"][:rows, noff:noff + nsz], psc[:rows, :nsz],
                        AFT.Identity)
                    psc = psH.tile([128, 512], f32, name="psc2", tag="psh")
                    nc.tensor.matmul(psc[:rows, :nsz], ld[:, :rows],
                                     hw_sb["fct"][:, noff:noff + nsz],
                                     start=True, stop=True)
                    nc.scalar.activation(
                        stag["t"][:rows, noff:noff + nsz], psc[:rows, :nsz],
                        AFT.Identity)
                    psc = psH.tile([128, 512], f32, name="psc3", tag="psh")
                    nc.tensor.matmul(psc[:rows, :nsz], od[:, :rows],
                                     fce0[:, noff:noff + nsz],
                                     start=True, stop=False)
                    nc.tensor.matmul(psc[:rows, :nsz], oh[:, :rows],
                                     fce1[:, noff:noff + nsz],
                                     start=False, stop=True)
                    nc.vector.tensor_copy(
                        stag["e"][:rows, noff:noff + nsz], psc[:rows, :nsz])
                for nm in ("c", "t", "e"):
                    nc.sync.dma_start(out_flat[nm][128 * j:128 * j + rows, :],
                                      stag[nm][:rows, :])

            gru_scope = nc.named_scope("gru")
            gru_scope.__enter__()
            # the two GRUs' chains are emitted op-by-op interleaved: the
            # static per-engine queue order then matches dataflow order, so
            # neither chain head-of-line-blocks the other.
            for t in range(L):
                hp, xb, psrz, psn = {}, {}, {}, {}
                psg = psG.tile([EMB, 48], f32, name="psg", tag="psg")
                for u in range(2):
                    hp[u] = (outT[:, 16 * (t - 1) + 8 * u:
                                  16 * (t - 1) + 8 * u + 8]
                             if t > 0 else zero16[:, 8 * u:8 * u + 8])
                    xb[u] = 48 * t + 24 * u
                    psrz[u] = psg[:, 24 * u:24 * u + 16]
                    psn[u] = psg[:, 24 * u + 16:24 * u + 24]
                    # bias fold is h-independent: keep PE busy at step start
                    nc.tensor.matmul(psn[u], bhhn_sb[u][:], ones[:, 0:8],
                                     start=True, stop=False)
                for u in range(2):
                    nc.tensor.matmul(psrz[u][:, 0:8], whh_sb[u][:, 0:H],
                                     hp[u], start=True, stop=True)
                    nc.tensor.matmul(psrz[u][:, 8:16], whh_sb[u][:, H:2 * H],
                                     hp[u], start=True, stop=True)
                    nc.tensor.matmul(psn[u], whh_sb[u][:, 2 * H:3 * H], hp[u],
                                     start=False, stop=True)
                # both u's elementwise chain as single ops on 2-segment APs
                # (psg/xp per-step layout is [rz0 n0 | rz1 n1], stride 24)
                psg2 = psg.bitcast(f32r).rearrange("p (c s) -> p c s", c=2)
                xp2 = xp[:, 48 * t:48 * t + 48] \
                    .rearrange("p (c s) -> p c s", c=2)
                hpb = (outT[:, 16 * (t - 1):16 * (t - 1) + 16]
                       if t > 0 else zero16[:, 0:16])
                rzp = sbs.tile([EMB, 32], f32r, name="rzp", tag="rzp")
                rzp2 = rzp.rearrange("p (c s) -> p c s", c=2)
                nc.vector.tensor_add(rzp2, psg2[:, :, 0:16],
                                     xp2[:, :, 0:16])
                gates = sbs.tile([EMB, 32], bf16, name="gates", tag="gates")
                nc.scalar.activation(gates[:], rzp[:], AFT.Sigmoid)
                gates2 = gates.rearrange("p (c s) -> p c s", c=2)
                rn = sbs.tile([EMB, 16], f32r, name="rn", tag="rn")
                rn2 = rn.rearrange("p (c s) -> p c s", c=2)
                nc.vector.tensor_mul(rn2, psg2[:, :, 16:24],
                                     gates2[:, :, 0:8])
                npre = sbs.tile([EMB, 16], f32r, name="npre", tag="npre")
                nc.vector.tensor_add(npre.rearrange("p (c s) -> p c s", c=2),
                                     rn2, xp2[:, :, 16:24])
                nn = sbs.tile([EMB, 16], bf16, name="nn", tag="nn")
                nc.scalar.activation(nn[:], npre[:], AFT.Tanh)
                dd = sbs.tile([EMB, 16], bf16, name="dd", tag="dd")
                nc.vector.tensor_sub(dd[:], hpb, nn[:])
                zd = sbs.tile([EMB, 16], bf16, name="zd", tag="zd")
                nc.vector.tensor_mul(zd.rearrange("p (c s) -> p c s", c=2),
                                     gates2[:, :, 8:16],
                                     dd.rearrange("p (c s) -> p c s", c=2))
                nc.vector.tensor_add(outT[:, 16 * t:16 * t + 16],
                                     nn[:], zd[:])
            gru_scope.__exit__(None, None, None)
            # heads emitted after the loop: lower scheduler priority, so the
            # recurrence chain never waits behind head matmuls
            with nc.named_scope("heads"):
                for j in range(12):
                    head_chunk(j, 16)
                head_chunk(12, 8)  # last 64 rows (t in [192,200))

            if debug:
                nc.sync.dma_start(dbg["outT"][:], outT[:])

    nc.compile()
    return nc


def _host_prep(inputs):
    """Build the 8 per-core input maps from the full problem inputs."""
    from concourse import mybir
    f = np.float32
    bf = mybir.dt.np(mybir.dt.bfloat16)
    x = inputs["x"].astype(f, copy=False)
    ques = inputs["ques"].astype(f, copy=False)

    def T(a, dt=None):
        return np.ascontiguousarray(
            np.asarray(a).T.astype(dt or f, copy=False))

    # layer-1 GCN activations, computed on host (tiny)
    z1 = {"hg": ques @ inputs["hg_W1"] + inputs["hg_b1"],
          "g1": ques @ inputs["g1_W1"] + inputs["g1_b1"],
          "g2": ques @ inputs["g2_W1"] + inputs["g2_b1"]}
    graphs = {"hg": inputs["G"], "g1": inputs["adj_out"], "g2": inputs["adj_in"]}

    def kperm(a, w):
        # [NQ, w] -> [KC, NK*w] with arr2[p, k*w+j] = arr[k*KC+p, j]
        return np.ascontiguousarray(
            np.asarray(a).reshape(NK, KC, w).transpose(1, 0, 2)
            .reshape(KC, NK * w))

    shared = {
        "z1_hg": kperm(np.asarray(z1["hg"]).astype(bf), EMB),
        "z1_g1": kperm(np.asarray(z1["g1"]).astype(bf), EMB),
        "z1_g2": kperm(np.asarray(z1["g2"]).astype(bf), EMB),
        "w2_hg": np.ascontiguousarray(np.asarray(inputs["hg_W2"]).astype(bf)),
        "w2_g1": np.ascontiguousarray(np.asarray(inputs["g1_W2"]).astype(bf)),
        "w2_g2": np.ascontiguousarray(np.asarray(inputs["g2_W2"]).astype(bf)),
        "b2_hg": np.asarray(inputs["hg_b2"]).astype(bf).reshape(1, -1),
        "b2_g1": np.asarray(inputs["g1_b2"]).astype(bf).reshape(1, -1),
        "b2_g2": np.asarray(inputs["g2_b2"]).astype(bf).reshape(1, -1),
        "wihT1": T(inputs["r1_Wih"], bf),
        "wihT2": T(inputs["r2_Wih"], bf),
        "whhT1": T(inputs["r1_Whh"], bf),
        "whhT2": T(inputs["r2_Whh"], bf),
        "w1wT": T(inputs["w1_W"], bf),
        "w2wT": T(inputs["w2_W"], bf),
        "wb": np.asarray(inputs["w1_b"] + inputs["w2_b"], f).reshape(-1, 1),
        "fccwT": T(inputs["fcc_W"], bf),
        "fctwT": T(inputs["fct_W"], bf),
        "fcewT": T(inputs["fce_W"], bf),
    }
    for u, (ih, hh) in enumerate((("r1_bih", "r1_bhh"), ("r2_bih", "r2_bhh"))):
        bih = np.asarray(inputs[ih], f)
        bhh = np.asarray(inputs[hh], f)
        pb = np.zeros((EMB, 3), f)
        for g in range(3):
            pb[:, g] = bih[g * H:(g + 1) * H]
            if g < 2:  # r, z: fold bhh into the projection bias
                pb[:, g] += bhh[g * H:(g + 1) * H]
        shared[f"projb{u + 1}"] = pb
        shared[f"bhhn{u + 1}"] = \
            bhh[2 * H:3 * H].reshape(1, -1).astype(bf).copy()

    in_maps = []
    for c in range(NCORES):
        m = dict(shared)
        for g, arr in graphs.items():
            blk = np.asarray(arr)[c * SHARD:(c + 1) * SHARD, :]
            atc = np.zeros((NQ, SHARD_P), bf)
            atc[:, :SHARD] = blk.astype(f, copy=False).T.astype(bf)
            m[f"at_{g}"] = kperm(atc, SHARD_P)
        xc = x[c * BLOC:(c + 1) * BLOC]           # [8, 200, 5000]
        m["xt"] = kperm(
            xc.transpose(2, 1, 0).reshape(NQ, BLC).astype(bf), BLC)
        in_maps.append(m)
    return in_maps


def kernel(**inputs):
    global _BUILT, LAST
    from concourse import bass_utils
    if _BUILT is None:
        _BUILT = _build(debug=False)
    nc = _BUILT
    in_maps = _host_prep(inputs)
    res = bass_utils.run_bass_kernel_spmd(nc, in_maps,
                                          core_ids=list(range(NCORES)))
    LAST = res
    f = np.float32
    logit_c = np.empty((B, L, Q), f)
    logit_t = np.empty((B, L, Q), f)
    logit_e = np.empty((B, L, Q), f)
    for c in range(NCORES):
        r = res.results[c]
        logit_c[c * BLOC:(c + 1) * BLOC] = \
            r["out_c"].astype(f).transpose(1, 0, 2)
        logit_t[c * BLOC:(c + 1) * BLOC] = \
            r["out_t"].astype(f).transpose(1, 0, 2)
        logit_e[c * BLOC:(c + 1) * BLOC] = \
            r["out_e"].astype(f).transpose(1, 0, 2)
    for arr, bname in ((logit_c, "fcc_b"), (logit_t, "fct_b"),
                       (logit_e, "fce_b")):
        bias = np.asarray(inputs[bname], f)
        if np.any(bias):
            arr += bias
    return (logit_c, logit_t, logit_e)



# revision 78
# speedup vs baseline: 1.4426x; 1.1092x over previous
"""Trainium2 Bass kernel for the DKT (graph-based knowledge tracing) model.

Sharding across the 8 NeuronCores:
  - GCN phase: row-shard of the three [5000,5000] adjacency matmuls (625 rows
    per core), with AllGathers of the small [5000,EMB] intermediates.
  - x@ques / GRU / logit heads: data-parallel over batch (8 sequences/core).

All layouts are chosen so every matmul contracts over the partition dim with
contiguous DMA: the host passes A.T column-shards, x.T (time-major columns)
shards, host-computed layer-1 GCN activations, and transposed weights.
"""

import numpy as np

Q = 2500
NQ = 5000
EMB = 128
H = 128
B = 64
L = 200
NCORES = 8
SHARD = NQ // NCORES          # 625 adjacency rows per core
KC = 125                      # contraction chunk (partition dim)
NK = NQ // KC                 # 40 chunks
BLOC = B // NCORES            # 8 sequences per core
BLC = L * BLOC                # 1600 (bl index = t*8 + b, t-major)
SHARD_P = 640                 # shard padded to even halves (fp32r ISA rule)
NH = [(0, 320), (320, 320)]   # padded-shard column halves (>=256, even)
XNT = [(0, 512), (512, 512), (1024, 512), (1536, 64)]  # x-stage N tiles
HNT = [(0, 512), (512, 512), (1024, 512), (1536, 512), (2048, 452)]

_BUILT = None
LAST = None


def _build(debug=False):
    import concourse.bass as bass  # noqa: F401
    import concourse.tile as tile
    from concourse import bacc, mybir
    from concourse.masks import make_identity
    from contextlib import ExitStack

    f32 = mybir.dt.float32
    f32r = mybir.dt.float32r
    bf16 = mybir.dt.bfloat16
    AFT = mybir.ActivationFunctionType
    ALU = mybir.AluOpType

    nc = bacc.Bacc("TRN2", target_bir_lowering=False, debug=False,
                   num_devices=NCORES)

    def din(name, shape, dt=f32r):
        return nc.dram_tensor(name, shape, dt, kind="ExternalInput").ap()

    def dout(name, shape, dt=f32):
        return nc.dram_tensor(name, shape, dt, kind="ExternalOutput").ap()

    # --- inputs (per-core unless noted) ---
    # at/xt/z1 come host-permuted to k-chunk-contiguous per partition:
    # arr2[p, k*W + j] = arr[k*KC + p, j] so every DMA is a plain
    # contiguous [125, N] slice (SWDGE descriptor-gen cost was a
    # bottleneck for the strided "(k p) -> p k" views).
    at = {g: din(f"at_{g}", [KC, NK * SHARD_P], bf16)
          for g in ("hg", "g1", "g2")}
    xt = din("xt", [KC, NK * BLC], bf16)
    z1 = {g: din(f"z1_{g}", [KC, NK * EMB], bf16) for g in ("hg", "g1", "g2")}
    e2s = {"hg": EMB, "g1": EMB // 2, "g2": EMB // 2}
    w2 = {g: din(f"w2_{g}", [EMB, e2s[g]], bf16) for g in ("hg", "g1", "g2")}
    b2 = {g: din(f"b2_{g}", [1, e2s[g]], bf16) for g in ("hg", "g1", "g2")}
    wihT = [din("wihT1", [EMB, 3 * H], bf16), din("wihT2", [EMB, 3 * H], bf16)]
    whhT = [din("whhT1", [EMB, 3 * H], bf16), din("whhT2", [EMB, 3 * H], bf16)]
    projb = [din("projb1", [EMB, 3], f32), din("projb2", [EMB, 3], f32)]
    bhhn = [din("bhhn1", [1, EMB], bf16), din("bhhn2", [1, EMB], bf16)]
    w1wT = din("w1wT", [EMB, EMB], bf16)
    w2wT = din("w2wT", [EMB, EMB], bf16)
    wb = din("wb", [EMB, 1], f32)
    fccwT = din("fccwT", [EMB, Q], bf16)
    fctwT = din("fctwT", [EMB, Q], bf16)
    fcewT = din("fcewT", [2 * EMB, Q], bf16)

    out_c = dout("out_c", [L, BLOC, Q], bf16)
    out_t = dout("out_t", [L, BLOC, Q], bf16)
    out_e = dout("out_e", [L, BLOC, Q], bf16)
    dbg = {}
    if debug:
        dbg["qh"] = dout("dbg_qh", [NQ, EMB], bf16)
        dbg["qd"] = dout("dbg_qd", [NQ, EMB], bf16)
        dbg["xh"] = dout("dbg_xh", [EMB, BLC], bf16)
        dbg["xd"] = dout("dbg_xd", [EMB, BLC], bf16)
        dbg["xp"] = dout("dbg_xp", [EMB, L * 48], bf16)
        dbg["outT"] = dout("dbg_outT", [EMB, L * 16], bf16)

    with tile.TileContext(nc) as tc, ExitStack() as ctx:
        const = ctx.enter_context(tc.tile_pool(name="const", bufs=1))
        dram = ctx.enter_context(tc.tile_pool(name="dram", bufs=1, space="DRAM"))

        ident = const.tile([128, 128], f32, name="ident")
        make_identity(nc, ident[:])
        ones_f = const.tile([1, 128], f32, name="ones_f")
        nc.gpsimd.memset(ones_f[:], 1.0)
        ones = const.tile([1, 128], bf16, name="ones")
        nc.vector.tensor_copy(ones[:], ones_f[:])
        ident_r = const.tile([128, 128], f32r, name="ident_r")
        nc.vector.tensor_copy(ident_r[:], ident[:])
        ident_b = const.tile([128, 128], bf16, name="ident_b")
        nc.vector.tensor_copy(ident_b[:], ident[:])

        # DRAM bounce buffers for the AllGathers, kept p-major ([KC, 5*EMB]
        # per core, concat over cores) so both the staging store and the
        # gathered load are contiguous per partition
        W5 = 5 * EMB
        zb = {"hg": dram.tile([KC, W5], bf16, name="zb_hg"),
              "pr": dram.tile([KC, W5], bf16, name="zb_pr")}
        zf = {"hg": dram.tile([NCORES * KC, W5], bf16, name="zf_hg",
                              addr_space="Shared"),
              "pr": dram.tile([NCORES * KC, W5], bf16, name="zf_pr",
                              addr_space="Shared")}
        qb = {"hg": dram.tile([KC, W5], bf16, name="qb_hg"),
              "pr": dram.tile([KC, W5], bf16, name="qb_pr")}
        qf = {"hg": dram.tile([NCORES * KC, W5], bf16, name="qf_hg",
                              addr_space="Shared"),
              "pr": dram.tile([NCORES * KC, W5], bf16, name="qf_pr",
                              addr_space="Shared")}
        RG = [list(range(NCORES))]

        def allgather(inb, outb):
            nc.gpsimd.collective_compute(
                "AllGather", ALU.bypass, replica_groups=RG,
                ins=[inb.opt()], outs=[outb.opt()])

        def rearr_kpe(ap, e):
            return ap.rearrange("(k p) e -> p k e", p=KC)

        # SWDGE (gpsimd) sprays one dma_start across all 16 SDMA engines;
        # a sync-queue (HWDGE) dma runs on a single engine (~36 GB/s) and
        # serializes. All bulk streams go through gpsimd.
        def big_dma(out, in_):
            nc.gpsimd.dma_start(out, in_)

        # ================= GCN phase =================
        # x-phase pools are allocated UP FRONT (disjoint SBUF/PSUM from the
        # GCN pools) so the x@qh pass can overlap the GCN tail instead of
        # inheriting write-after-read deps from address reuse.
        sbQ = ctx.enter_context(tc.tile_pool(name="sbQ", bufs=1))
        qh_sb = sbQ.tile([KC, NK * EMB], bf16, name="qh_sb")
        qd_sb = sbQ.tile([KC, NK * EMB], bf16, name="qd_sb")
        sbP = ctx.enter_context(tc.tile_pool(name="sbP", bufs=1))
        xp = sbP.tile([EMB, L * 48], bf16, name="xp")
        xp_v = xp.rearrange("p (t u g b) -> p t u g b", u=2, g=3, b=BLOC)
        sbX = ctx.enter_context(tc.tile_pool(name="sbX", bufs=1))
        xhT = sbX.tile([EMB, BLC], bf16, name="xhT")
        xdT = sbX.tile([EMB, BLC], bf16, name="xdT")
        xstream = ctx.enter_context(tc.tile_pool(name="xstream", bufs=2))
        with tc.tile_pool(name="sbG", bufs=1) as sbG, \
             tc.tile_pool(name="astream", bufs=2) as astream, \
             tc.tile_pool(name="psA", bufs=2, space="PSUM") as psA, \
             tc.tile_pool(name="psW", bufs=1, space="PSUM") as psW, \
             tc.tile_pool(name="psT", bufs=1, space="PSUM") as psT:

            z1sb, hT, w2sb, b2sb = {}, {}, {}, {}
            zstag, qstag, z2f = {}, {}, {}
            ahold = {}
            APIECE = [(0, 8), (8, 16), (24, 16)]   # k-chunk DMA pieces

            def gcn_stage1(g):
                e2 = e2s[g]
                z1sb[g] = sbG.tile([KC, NK * EMB], bf16, name=f"z1sb_{g}", tag="z1sb", bufs=1)
                big_dma(z1sb[g][:], z1[g][:])
                w2sb[g] = sbG.tile([EMB, e2], bf16, name=f"w2sb_{g}")
                nc.sync.dma_start(w2sb[g][:], w2[g][:])
                b2sb[g] = sbG.tile([1, e2], bf16, name=f"b2sb_{g}")
                nc.sync.dma_start(b2sb[g][:], b2[g][:])
                hT[g] = sbG.tile([EMB, SHARD_P], bf16, name=f"hT_{g}", tag="hT", bufs=2)

                # stream at_g ONCE into a held buffer; stage2a re-reads it
                # from SBUF (at traffic halved: DMA is the phase roofline)
                a_t = astream.tile([KC, NK * SHARD_P], bf16, name=f"a_{g}",
                                   tag="a")
                ahold[g] = a_t
                for k0, nk_ in APIECE:
                    big_dma(a_t[:, k0 * SHARD_P:(k0 + nk_) * SHARD_P],
                            at[g][:, k0 * SHARD_P:(k0 + nk_) * SHARD_P])
                ps = [psA.tile([EMB, 512], f32, name=f"ps1_{g}{i}", tag="psA")
                      for i in range(2)]
                for k in range(NK):
                    for i, (off, nh) in enumerate(NH):
                        nc.tensor.matmul(
                            ps[i][:, :nh],
                            z1sb[g][:, k * EMB:(k + 1) * EMB],
                            a_t[:, k * SHARD_P + off:k * SHARD_P + off + nh],
                            start=(k == 0), stop=(k == NK - 1))
                for i, (off, nh) in enumerate(NH):
                    nc.scalar.activation(hT[g][:, off:off + nh],
                                         ps[i][:EMB, :nh], AFT.Relu)

            def gcn_stage2w(g, grp, coloff):
                # Z2 = h @ W2 + b2 (natural layout, into the AG input staging)
                e2 = e2s[g]
                if grp not in zstag:
                    zstag[grp] = sbG.tile([KC, 5 * EMB], bf16,
                                          name=f"zstag_{grp}")
                for c in range(5):
                    ps = psW.tile([KC, EMB], f32, name="psW", tag="psW")
                    nc.tensor.matmul(ps[:, :e2], hT[g][:, c * KC:(c + 1) * KC],
                                     w2sb[g][:], start=True, stop=False)
                    nc.tensor.matmul(ps[:, :e2], ones[:, :KC], b2sb[g][:],
                                     start=False, stop=True)
                    nc.vector.tensor_copy(
                        zstag[grp][:, c * EMB + coloff: c * EMB + coloff + e2],
                        ps[:, :e2])

            def ag_z(grp):
                nc.sync.dma_start(zb[grp][:], zstag[grp][:])
                allgather(zb[grp], zf[grp])
                z2f[grp] = sbG.tile([KC, NK * EMB], bf16, name=f"z2f_{grp}", tag="z2f", bufs=1)
                nc.gpsimd.dma_start(
                    z2f[grp].rearrange("p (c w) -> p c w", c=NCORES),
                    zf[grp].rearrange("(c p) w -> p c w", p=KC))

            def gcn_stage2a(g, grp, coloff):
                e2 = e2s[g]
                o2T = sbG.tile([e2, SHARD_P], f32, name=f"o2T_{g}", tag="o2T", bufs=2)
                ps = [psA.tile([EMB, 512], f32, name=f"ps2_{g}{i}", tag="psA")
                      for i in range(2)]
                a_t = ahold[g]
                for k in range(NK):
                    for i, (off, nh) in enumerate(NH):
                        nc.tensor.matmul(
                            ps[i][:e2, :nh],
                            z2f[grp][:, k * EMB + coloff: k * EMB + coloff + e2],
                            a_t[:, k * SHARD_P + off:k * SHARD_P + off + nh],
                            start=(k == 0), stop=(k == NK - 1))
                for i, (off, nh) in enumerate(NH):
                    nc.vector.tensor_copy(o2T[:, off:off + nh], ps[i][:e2, :nh])
                # transpose to natural layout; stage for the output AllGather
                if grp not in qstag:
                    qstag[grp] = sbG.tile([KC, 5 * EMB], bf16,
                                          name=f"qstag_{grp}")
                # ques_d = concat([ques_in(g2), ques_out(g1)]): g2 -> cols
                # 0:64, g1 -> cols 64:128 of each block; hg -> full block.
                qoff = {"hg": 0, "g1": 64, "g2": 0}[g]
                for c in range(5):
                    pst = psT.tile([KC, EMB], f32, name="psT", tag="psT")
                    nc.tensor.transpose(pst[:, :e2],
                                        o2T[:, c * KC:(c + 1) * KC],
                                        ident[:e2, :e2])
                    nc.vector.tensor_copy(
                        qstag[grp][:, c * EMB + qoff: c * EMB + qoff + e2],
                        pst[:, :e2])

            def ag_q(grp):
                nc.sync.dma_start(qb[grp][:], qstag[grp][:])
                allgather(qb[grp], qf[grp])

            with nc.named_scope("gcn"):
                gcn_stage1("hg")
                gcn_stage2w("hg", "hg", 0)
                ag_z("hg")
                gcn_stage1("g1")
                gcn_stage2a("hg", "hg", 0)
                ag_q("hg")
                nc.gpsimd.dma_start(
                    qh_sb.rearrange("p (c w) -> p c w", c=NCORES),
                    qf["hg"].rearrange("(c p) w -> p c w", p=KC))
                gcn_stage2w("g1", "pr", 0)
                gcn_stage1("g2")
                gcn_stage2w("g2", "pr", 64)
                ag_z("pr")
                gcn_stage2a("g1", "pr", 0)
                gcn_stage2a("g2", "pr", 64)
                ag_q("pr")
                nc.gpsimd.dma_start(
                    qd_sb.rearrange("p (c w) -> p c w", c=NCORES),
                    qf["pr"].rearrange("(c p) w -> p c w", p=KC))

        if debug:
            for nm, src in (("qh", qf["hg"]), ("qd", qf["pr"])):
                nc.sync.dma_start(
                    dbg[nm].rearrange("(c kk p) e -> p c kk e",
                                      p=KC, c=NCORES),
                    src.rearrange("(c p) (kk e) -> p c kk e", p=KC, e=EMB))

        # ================= x @ ques phase =================
        XQ = 2                # k-chunks per stream piece
        psX_cm = tc.tile_pool(name="psX", bufs=1, space="PSUM")
        psX = psX_cm.__enter__()
        with nc.named_scope("xques"):
            psh = [psX.tile([EMB, 512], f32, name=f"psxh{i}", tag=f"psxh{i}")
                   for i in range(4)]
            psd = [psX.tile([EMB, 512], f32, name=f"psxd{i}", tag=f"psxd{i}")
                   for i in range(4)]
            for q in range(NK // XQ):
                xsb = xstream.tile([KC, XQ * BLC], bf16, name="xsb",
                                   tag="xsb")
                big_dma(xsb[:], xt[:, q * XQ * BLC:(q + 1) * XQ * BLC])
                for kk in range(XQ):
                    k = q * XQ + kk
                    for i, (off, nn_) in enumerate(XNT):
                        nc.tensor.matmul(psh[i][:, :nn_],
                                         qh_sb[:, k * EMB:(k + 1) * EMB],
                                         xsb[:, kk * BLC + off:kk * BLC + off + nn_],
                                         start=(k == 0), stop=(k == NK - 1))
                        nc.tensor.matmul(psd[i][:, :nn_],
                                         qd_sb[:, k * EMB:(k + 1) * EMB],
                                         xsb[:, kk * BLC + off:kk * BLC + off + nn_],
                                         start=(k == 0), stop=(k == NK - 1))
            for i, (off, nn_) in enumerate(XNT):
                nc.vector.tensor_copy(xhT[:, off:off + nn_], psh[i][:, :nn_])
                nc.vector.tensor_copy(xdT[:, off:off + nn_], psd[i][:, :nn_])

        if debug:
            nc.sync.dma_start(dbg["xh"][:], xhT[:])
            nc.sync.dma_start(dbg["xd"][:], xdT[:])
        psX_cm.__exit__(None, None, None)

        # ============ GRU input projections ============
        # xp column layout per step t: [xr1 xr2 xz1 xz2 xn1 xn2] (8 each)
        with tc.tile_pool(name="psP", bufs=3, space="PSUM") as psP, \
             tc.tile_pool(name="sbW", bufs=1) as sbW, \
             nc.named_scope("proj"):
            wih_sb, pb_sb = [], []
            for u in range(2):
                wt = sbW.tile([EMB, 3 * H], bf16, name=f"wihsb{u}")
                nc.sync.dma_start(wt[:], wihT[u][:])
                wih_sb.append(wt)
                pb = sbW.tile([EMB, 3], f32, name=f"pbsb{u}")
                nc.sync.dma_start(pb[:], projb[u][:])
                pb_sb.append(pb)
            # nt-major so the GRU's first steps unblock after the first tile
            for nt in range(4):
                for u in range(2):
                    src = xhT if u == 0 else xdT
                    for g in range(3):
                        ps = psP.tile([EMB, 400], f32, name="psP",
                                      tag="psP")
                        nc.tensor.matmul(
                            ps[:], wih_sb[u][:, g * H:(g + 1) * H],
                            src[:, nt * 400:(nt + 1) * 400],
                            start=True, stop=True)
                        nc.scalar.activation(
                            xp_v[:, nt * 50:(nt + 1) * 50, u, g, :],
                            ps.rearrange("p (t b) -> p t b", b=BLOC),
                            AFT.Identity, bias=pb_sb[u][:, g:g + 1])
        if debug:
            nc.sync.dma_start(dbg["xp"][:], xp[:])

        # ================= GRU + heads phase =================
        with tc.tile_pool(name="sbR", bufs=1) as sbR, \
             tc.tile_pool(name="sbh", bufs=2) as sbh, \
             tc.tile_pool(name="sbstep", bufs=4) as sbs, \
             tc.tile_pool(name="stg", bufs=2) as stg, \
             tc.tile_pool(name="psG", bufs=5, space="PSUM") as psG, \
             tc.tile_pool(name="psTh", bufs=1, space="PSUM") as psTh, \
             tc.tile_pool(name="psH", bufs=2, space="PSUM") as psH:
            whh_sb, bhhn_sb = [], []
            for u in range(2):
                wt = sbR.tile([EMB, 3 * H], bf16, name=f"whhsb{u}")
                nc.sync.dma_start(wt[:], whhT[u][:])
                whh_sb.append(wt)
                bt = sbR.tile([1, EMB], bf16, name=f"bhhnsb{u}")
                nc.sync.dma_start(bt[:], bhhn[u][:])
                bhhn_sb.append(bt)
            w1w_sb = sbR.tile([EMB, EMB], bf16, name="w1wsb")
            nc.sync.dma_start(w1w_sb[:], w1wT[:])
            w2w_sb = sbR.tile([EMB, EMB], bf16, name="w2wsb")
            nc.sync.dma_start(w2w_sb[:], w2wT[:])
            wb_sb = sbR.tile([EMB, 1], f32, name="wbsb")
            nc.sync.dma_start(wb_sb[:], wb[:])
            # head weights are not needed until ~the GRU: delay their DMA so
            # they don't steal HBM bandwidth from the GCN A-streams
            hw_sb = {}
            with tc.tile_wait_until(0.35):
                for nm, t_ in (("fcc", fccwT), ("fct", fctwT)):
                    w_ = sbR.tile([EMB, Q], bf16, name=f"{nm}wsb")
                    nc.sync.dma_start(w_[:], t_[:])
                    hw_sb[nm] = w_
                fce0 = sbR.tile([EMB, Q], bf16, name="fce0sb")
                nc.sync.dma_start(fce0[:], fcewT[0:EMB, :])
                fce1 = sbR.tile([EMB, Q], bf16, name="fce1sb")
                nc.sync.dma_start(fce1[:], fcewT[EMB:2 * EMB, :])

            outT = sbR.tile([EMB, L * 16], bf16, name="outT")
            outT_v = outT.rearrange("p (t u b) -> p t u b", u=2, b=BLOC)
            zero16_f = sbR.tile([EMB, 16], f32, name="zero16_f")
            nc.gpsimd.memset(zero16_f[:], 0.0)
            zero16 = sbR.tile([EMB, 16], bf16, name="zero16")
            nc.vector.tensor_copy(zero16[:], zero16_f[:])
            stag = {nm: stg.tile([128, Q], bf16, name=f"stag_{nm}")
                    for nm in ("c", "t", "e")}
            out_flat = {"c": out_c.rearrange("l b q -> (l b) q"),
                        "t": out_t.rearrange("l b q -> (l b) q"),
                        "e": out_e.rearrange("l b q -> (l b) q")}

            def head_chunk(j, nt16):
                rows = nt16 * BLOC
                lh = sbh.tile([EMB, 128], bf16, name="lh", tag="lh")
                ld = sbh.tile([EMB, 128], bf16, name="ld", tag="ld")
                nc.vector.tensor_copy(
                    lh[:, :rows].rearrange("p (t b) -> p t b", b=BLOC),
                    outT_v[:, 16 * j:16 * j + nt16, 0, :])
                nc.vector.tensor_copy(
                    ld[:, :rows].rearrange("p (t b) -> p t b", b=BLOC),
                    outT_v[:, 16 * j:16 * j + nt16, 1, :])
                pst = psTh.tile([EMB, 128], f32, name="pstheta", tag="pstheta")
                nc.tensor.matmul(pst[:, :rows], w1w_sb[:], lh[:, :rows],
                                 start=True, stop=False)
                nc.tensor.matmul(pst[:, :rows], w2w_sb[:], ld[:, :rows],
                                 start=False, stop=True)
                theta = sbh.tile([EMB, 128], bf16, name="theta", tag="theta")
                nc.scalar.activation(theta[:, :rows], pst[:, :rows],
                                     AFT.Sigmoid, bias=wb_sb[:])
                omt = sbh.tile([EMB, 128], bf16, name="omt", tag="omt")
                nc.scalar.activation(omt[:, :rows], theta[:, :rows],
                                     AFT.Identity, scale=-1.0, bias=1.0)
                od = sbh.tile([EMB, 128], bf16, name="od", tag="od")
                nc.vector.tensor_mul(od[:, :rows], theta[:, :rows],
                                     ld[:, :rows])
                oh = sbh.tile([EMB, 128], bf16, name="oh", tag="oh")
                nc.vector.tensor_mul(oh[:, :rows], omt[:, :rows],
                                     lh[:, :rows])
                for noff, nsz in HNT:
                    psc = psH.tile([128, 512], f32, name="psc", tag="psh")
                    nc.tensor.matmul(psc[:rows, :nsz], lh[:, :rows],
                                     hw_sb["fcc"][:, noff:noff + nsz],
                                     start=True, stop=True)
                    nc.scalar.activation(
                        stag["c"][:rows, noff:noff + nsz], psc[:rows, :nsz],
                        AFT.Identity)
                    psc = psH.tile([128, 512], f32, name="psc2", tag="psh")
                    nc.tensor.matmul(psc[:rows, :nsz], ld[:, :rows],
                                     hw_sb["fct"][:, noff:noff + nsz],
                                     start=True, stop=True)
                    nc.scalar.activation(
                        stag["t"][:rows, noff:noff + nsz], psc[:rows, :nsz],
                        AFT.Identity)
                    psc = psH.tile([128, 512], f32, name="psc3", tag="psh")
                    nc.tensor.matmul(psc[:rows, :nsz], od[:, :rows],
                                     fce0[:, noff:noff + nsz],
                                     start=True, stop=False)
                    nc.tensor.matmul(psc[:rows, :nsz], oh[:, :rows],
                                     fce1[:, noff:noff + nsz],
                                     start=False, stop=True)
                    nc.vector.tensor_copy(
                        stag["e"][:rows, noff:noff + nsz], psc[:rows, :nsz])
                for nm in ("c", "t", "e"):
                    nc.sync.dma_start(out_flat[nm][128 * j:128 * j + rows, :],
                                      stag[nm][:rows, :])

            gru_scope = nc.named_scope("gru")
            gru_scope.__enter__()
            # the two GRUs' chains are emitted op-by-op interleaved: the
            # static per-engine queue order then matches dataflow order, so
            # neither chain head-of-line-blocks the other.
            for t in range(L):
                # psg per-step layout: [rz0(16) rz1(16) n0(8) n1(8)]
                hp, psrz, psn = {}, {}, {}
                psg = psG.tile([EMB, 48], f32, name="psg", tag="psg")
                xp2 = xp[:, 48 * t:48 * t + 48] \
                    .rearrange("p (c s) -> p c s", c=2)
                hpb = (outT[:, 16 * (t - 1):16 * (t - 1) + 16]
                       if t > 0 else zero16[:, 0:16])
                for u in range(2):
                    hp[u] = hpb[:, 8 * u:8 * u + 8]
                    psrz[u] = psg[:, 16 * u:16 * u + 16]
                    psn[u] = psg[:, 32 + 8 * u:40 + 8 * u]
                    # bias fold is h-independent: keep PE busy at step start
                    nc.tensor.matmul(psn[u], bhhn_sb[u][:], ones[:, 0:8],
                                     start=True, stop=False,
                                     skip_group_check=True)
                # xp_rz folded into PSUM via identity (h-independent too):
                # the sigmoid then reads PSUM directly, no vector add hop
                nc.tensor.matmul(psg[:, 0:32], ident_b[:], xp2[:, :, 0:16],
                                 start=True, stop=False,
                                 skip_group_check=True)
                for u in range(2):
                    nc.tensor.matmul(psrz[u][:, 0:8], whh_sb[u][:, 0:H],
                                     hp[u], start=False, stop=True,
                                     skip_group_check=True)
                    nc.tensor.matmul(psrz[u][:, 8:16], whh_sb[u][:, H:2 * H],
                                     hp[u], start=False, stop=True,
                                     skip_group_check=True)
                    nc.tensor.matmul(psn[u], whh_sb[u][:, 2 * H:3 * H], hp[u],
                                     start=False, stop=True,
                                     skip_group_check=True)
                # merged elementwise chain (2-segment APs where u0/u1 data
                # is non-adjacent); tail is 2 hops: omz/zh precomputed
                gates = sbs.tile([EMB, 32], bf16, name="gates", tag="gates")
                nc.scalar.activation(gates[:], psg[:, 0:32], AFT.Sigmoid)
                gates2 = gates.rearrange("p (c s) -> p c s", c=2)
                rn = sbs.tile([EMB, 16], f32r, name="rn", tag="rn")
                rn2 = rn.rearrange("p (c s) -> p c s", c=2)
                nc.vector.tensor_mul(rn2, psg.bitcast(f32r)[:, 32:48]
                                     .rearrange("p (c s) -> p c s", c=2),
                                     gates2[:, :, 0:8])
                npre = sbs.tile([EMB, 16], f32r, name="npre", tag="npre")
                nc.vector.tensor_add(npre.rearrange("p (c s) -> p c s", c=2),
                                     rn2, xp2[:, :, 16:24])
                omz = sbs.tile([EMB, 16], bf16, name="omz", tag="omz")
                nc.vector.tensor_scalar(omz.rearrange("p (c s) -> p c s",
                                                      c=2),
                                        gates2[:, :, 8:16], -1.0, 1.0,
                                        ALU.mult, ALU.add)
                zh = sbs.tile([EMB, 16], bf16, name="zh", tag="zh")
                nc.vector.tensor_mul(zh.rearrange("p (c s) -> p c s", c=2),
                                     gates2[:, :, 8:16],
                                     hpb.rearrange("p (c s) -> p c s", c=2))
                nn = sbs.tile([EMB, 16], bf16, name="nn", tag="nn")
                nc.scalar.activation(nn[:], npre[:], AFT.Tanh)
                t1 = sbs.tile([EMB, 16], bf16, name="t1", tag="t1")
                nc.vector.tensor_mul(t1[:], nn[:], omz[:])
                nc.vector.tensor_add(outT[:, 16 * t:16 * t + 16],
                                     t1[:], zh[:])
            gru_scope.__exit__(None, None, None)
            # heads emitted after the loop: lower scheduler priority, so the
            # recurrence chain never waits behind head matmuls
            with nc.named_scope("heads"):
                for j in range(12):
                    head_chunk(j, 16)
                head_chunk(12, 8)  # last 64 rows (t in [192,200))

            if debug:
                nc.sync.dma_start(dbg["outT"][:], outT[:])

    nc.compile()
    return nc


def _host_prep(inputs):
    """Build the 8 per-core input maps from the full problem inputs."""
    from concourse import mybir
    f = np.float32
    bf = mybir.dt.np(mybir.dt.bfloat16)
    x = inputs["x"].astype(f, copy=False)
    ques = inputs["ques"].astype(f, copy=False)

    def T(a, dt=None):
        return np.ascontiguousarray(
            np.asarray(a).T.astype(dt or f, copy=False))

    # layer-1 GCN activations, computed on host (tiny)
    z1 = {"hg": ques @ inputs["hg_W1"] + inputs["hg_b1"],
          "g1": ques @ inputs["g1_W1"] + inputs["g1_b1"],
          "g2": ques @ inputs["g2_W1"] + inputs["g2_b1"]}
    graphs = {"hg": inputs["G"], "g1": inputs["adj_out"], "g2": inputs["adj_in"]}

    def kperm(a, w):
        # [NQ, w] -> [KC, NK*w] with arr2[p, k*w+j] = arr[k*KC+p, j]
        return np.ascontiguousarray(
            np.asarray(a).reshape(NK, KC, w).transpose(1, 0, 2)
            .reshape(KC, NK * w))

    shared = {
        "z1_hg": kperm(np.asarray(z1["hg"]).astype(bf), EMB),
        "z1_g1": kperm(np.asarray(z1["g1"]).astype(bf), EMB),
        "z1_g2": kperm(np.asarray(z1["g2"]).astype(bf), EMB),
        "w2_hg": np.ascontiguousarray(np.asarray(inputs["hg_W2"]).astype(bf)),
        "w2_g1": np.ascontiguousarray(np.asarray(inputs["g1_W2"]).astype(bf)),
        "w2_g2": np.ascontiguousarray(np.asarray(inputs["g2_W2"]).astype(bf)),
        "b2_hg": np.asarray(inputs["hg_b2"]).astype(bf).reshape(1, -1),
        "b2_g1": np.asarray(inputs["g1_b2"]).astype(bf).reshape(1, -1),
        "b2_g2": np.asarray(inputs["g2_b2"]).astype(bf).reshape(1, -1),
        "wihT1": T(inputs["r1_Wih"], bf),
        "wihT2": T(inputs["r2_Wih"], bf),
        "whhT1": T(inputs["r1_Whh"], bf),
        "whhT2": T(inputs["r2_Whh"], bf),
        "w1wT": T(inputs["w1_W"], bf),
        "w2wT": T(inputs["w2_W"], bf),
        "wb": np.asarray(inputs["w1_b"] + inputs["w2_b"], f).reshape(-1, 1),
        "fccwT": T(inputs["fcc_W"], bf),
        "fctwT": T(inputs["fct_W"], bf),
        "fcewT": T(inputs["fce_W"], bf),
    }
    for u, (ih, hh) in enumerate((("r1_bih", "r1_bhh"), ("r2_bih", "r2_bhh"))):
        bih = np.asarray(inputs[ih], f)
        bhh = np.asarray(inputs[hh], f)
        pb = np.zeros((EMB, 3), f)
        for g in range(3):
            pb[:, g] = bih[g * H:(g + 1) * H]
            if g < 2:  # r, z: fold bhh into the projection bias
                pb[:, g] += bhh[g * H:(g + 1) * H]
        shared[f"projb{u + 1}"] = pb
        shared[f"bhhn{u + 1}"] = \
            bhh[2 * H:3 * H].reshape(1, -1).astype(bf).copy()

    in_maps = []
    for c in range(NCORES):
        m = dict(shared)
        for g, arr in graphs.items():
            blk = np.asarray(arr)[c * SHARD:(c + 1) * SHARD, :]
            atc = np.zeros((NQ, SHARD_P), bf)
            atc[:, :SHARD] = blk.astype(f, copy=False).T.astype(bf)
            m[f"at_{g}"] = kperm(atc, SHARD_P)
        xc = x[c * BLOC:(c + 1) * BLOC]           # [8, 200, 5000]
        m["xt"] = kperm(
            xc.transpose(2, 1, 0).reshape(NQ, BLC).astype(bf), BLC)
        in_maps.append(m)
    return in_maps


def kernel(**inputs):
    global _BUILT, LAST
    from concourse import bass_utils
    if _BUILT is None:
        _BUILT = _build(debug=False)
    nc = _BUILT
    in_maps = _host_prep(inputs)
    res = bass_utils.run_bass_kernel_spmd(nc, in_maps,
                                          core_ids=list(range(NCORES)))
    LAST = res
    f = np.float32
    logit_c = np.empty((B, L, Q), f)
    logit_t = np.empty((B, L, Q), f)
    logit_e = np.empty((B, L, Q), f)
    for c in range(NCORES):
        r = res.results[c]
        logit_c[c * BLOC:(c + 1) * BLOC] = \
            r["out_c"].astype(f).transpose(1, 0, 2)
        logit_t[c * BLOC:(c + 1) * BLOC] = \
            r["out_t"].astype(f).transpose(1, 0, 2)
        logit_e[c * BLOC:(c + 1) * BLOC] = \
            r["out_e"].astype(f).transpose(1, 0, 2)
    for arr, bname in ((logit_c, "fcc_b"), (logit_t, "fct_b"),
                       (logit_e, "fce_b")):
        bias = np.asarray(inputs[bname], f)
        if np.any(bias):
            arr += bias
    return (logit_c, logit_t, logit_e)

